# revision 1
# baseline (speedup 1.0000x reference)
"""Trainium2 Bass kernel: bipartite GNN message passing (BranchingGNN), 8-core SPMD.

Sharding: core k owns constraint rows [k*6250,(k+1)*6250) and variable rows
[k*12500,(k+1)*12500); each core processes all edges targeting its shard, so
messages need no cross-core reduction. Node tables live row-major in DRAM
(bf16 features in the first 128B of a 256B-strided row) and are re-broadcast
each phase by an AllGather of the updated shards.

Per phase (one message direction):
  - edges sorted by destination; each destination's run is split by source
    window (int16 gather reach) and padded to 4-edge slots; slots are packed
    into 128-edge tiles per (dst-block, window), streamed window-major.
  - dma_gather (custom emit: 128B rows at 256B stride) fetches source rows,
    128 edges per partition-tile, up to 7 tiles per call (SWDGE ring cap).
  - stage 1: fixed one-hot S_fix [128,32] reduces each tile to 32 slot sums;
    4 tiles packed into one PSUM [128,64] via PE tile_position.
  - stage 2: one-hot S2 [128,128] (DVE iota==pdst compare, -1 pads give zero
    rows) turns pack partials into the block's msgT [64,128] contribution,
    accumulated into an SBUF msg buffer.
  - update: relu(h_prevT + W.T @ msgT + b) in transposed layout (per-partition
    bias on ACT); PE transpose back to row-major, DMA to shard, AllGather.
"""
import sys

sys.path.insert(0, "/opt/trn_rl_repo")

import numpy as np
import ml_dtypes

import concourse.bass as bass
import concourse.bacc as bacc
import concourse.mybir as mybir
import concourse.tile as tile
from concourse.bass_utils import run_bass_kernel_spmd

# ---- problem constants
V, C, E = 100000, 50000, 1250000
VF, CF, H = 32, 32, 64
ROUNDS = 3
CORES = 8
P = 128
WSLOT = 4             # edges per slot
SLOTS = P // WSLOT    # 32 slot-sums per 128-edge tile
TPP = 4               # tiles per pack (128 partials)
TPC = 7               # tiles per gather call (SWDGE ring cap 64 descs)
ROWB = 128            # table row width in bf16 elems (64 data + 64 pad = 256B)

V_CORE, C_CORE = 12500, 6250          # real nodes per core
V_S, C_S = 12672, 6400                # shard rows (99 / 50 blocks)
NBU_V, NBU_C = 98, 49                 # updated blocks (last block stays zero)
RV, RC = CORES * V_S, CORES * C_S     # 101376 / 51200 table rows
VWIN, CWIN = 4, 2                     # source windows (2 shards / 4 shards)
VWROWS, CWROWS = 2 * V_S, 4 * C_S     # 25344 / 25600 rows per window
VDUMMY, CDUMMY = 12544, 6272          # window-local zero row

BF16 = mybir.dt.bfloat16
F32 = mybir.dt.float32
I16 = mybir.dt.int16
BF = ml_dtypes.bfloat16


def _win_local(src, n_core, shard, per_win):
    """global node id -> (window, window-local table row)."""
    w = src // (per_win * n_core)
    local = (src % (per_win * n_core)) // n_core * shard + src % n_core
    return w, local


def _prep_direction(dst, src, n_dst_core, nblk_upd, nwin, src_core, src_shard,
                    dummy_row):
    """Metadata for one direction. Returns (idx16 [CORES,128,8*Ttot],
    pdst [CORES,128,Ptot], Tbw [nwin, nblk_upd])."""
    dst = np.asarray(dst, np.int64)
    src = np.asarray(src, np.int64)
    E_ = dst.size
    per_win_ids = src_core * (CORES // nwin)      # real ids per window

    core_of = dst // n_dst_core
    d_loc = dst % n_dst_core
    b_of = d_loc // P
    w_of = src // per_win_ids
    widx = (src % per_win_ids) // src_core * src_shard + src % src_core

    # sort edges by (window, core, block, dst)
    key = ((w_of * CORES + core_of) * nblk_upd + b_of) * P + d_loc % P
    order = np.argsort(key, kind="stable")
    ks = key[order]
    widx_s = widx[order]

    # run ranks within each (w, core, b, dst)
    counts = np.bincount(ks, minlength=nwin * CORES * nblk_upd * P)
    run_start = np.zeros(counts.size + 1, np.int64)
    run_start[1:] = np.cumsum(counts)
    rank = np.arange(E_, dtype=np.int64) - run_start[ks]

    slot_cnt = -(-counts // WSLOT)                                  # per key
    sc4 = slot_cnt.reshape(nwin, CORES, nblk_upd, P)
    blk_slots = sc4.sum(-1)                                         # [w,core,b]
    Tbw = np.maximum((-(-blk_slots // SLOTS)).max(1), 1)            # [w, b]

    # slot offset of each key within its (w, core, b) group
    sc_cum = np.cumsum(sc4, -1) - sc4                                # excl
    # tile base of (w, b): window-major, blocks in order (same every core)
    tiles_w = Tbw.sum(1)                                             # [w]
    win_base = np.zeros(nwin + 1, np.int64)
    win_base[1:] = np.cumsum(tiles_w)
    blk_base = np.cumsum(Tbw, 1) - Tbw                               # [w, b]
    Ttot = int(tiles_w.sum())

    w_s = ks // (CORES * nblk_upd * P)
    rem = ks % (CORES * nblk_upd * P)
    c_s = rem // (nblk_upd * P)
    b_s = rem % (nblk_upd * P) // P

    slot_pos = (win_base[w_s] + blk_base[w_s, b_s]) * SLOTS \
        + sc_cum.reshape(-1)[ks] + rank // WSLOT
    epos = slot_pos * WSLOT + rank % WSLOT

    idx16 = np.full((CORES, Ttot * P), dummy_row, np.int16)
    idx16[c_s, epos] = widx_s.astype(np.int16)

    # packs: per (w, b): ceil(Tbw/4); pdst flat slot -> pack/partial
    Pbw = -(-Tbw // TPP)                                             # [w, b]
    packs_w = Pbw.sum(1)
    pwin_base = np.zeros(nwin + 1, np.int64)
    pwin_base[1:] = np.cumsum(packs_w)
    pblk_base = np.cumsum(Pbw, 1) - Pbw
    Ptot = int(packs_w.sum())

    # slot position within its (w,b) group:
    s_in_blk = slot_pos - (win_base[w_s] + blk_base[w_s, b_s]) * SLOTS
    pack_of = pwin_base[w_s] + pblk_base[w_s, b_s] + s_in_blk // P
    part_of = s_in_blk % P
    pdst = np.full((CORES, Ptot, P), -1.0, np.float32)
    pdst[c_s, pack_of, part_of] = (ks % P).astype(np.float32)
    pdst = pdst.transpose(0, 2, 1).copy()                            # [CORES,128,Ptot]

    # idx16 -> dma_gather wrap layout [CORES, 128, 8*Ttot]
    packed = np.zeros((CORES, P, Ttot * 8), np.int16)
    for k in range(CORES):
        a = idx16[k].reshape(-1, 16).T                               # [16, Ttot*8]
        packed[k] = np.tile(a, (8, 1))
    return packed, pdst, Tbw.astype(int)



def _dma_gather_raw(gp, out_ap, in_ap, idxs_ap, num_idxs, elem_size, elem_step,
                    queue_num=0):
    """dma_gather (non-transpose, HBM source) allowing 128B rows at 256B stride."""
    from concourse import ap_utils
    gp._assert_queue_num(queue_num)
    assert idxs_ap.dtype == mybir.dt.int16
    assert in_ap.dtype == out_ap.dtype
    assert ap_utils.ap_is_contiguous(in_ap.ap[1:])
    assert ap_utils.ap_is_contiguous(out_ap.ap[1:])
    assert ap_utils.ap_is_contiguous(idxs_ap.ap[1:])
    assert in_ap.ap[-1][1] == out_ap.ap[-1][1] == elem_size
    assert out_ap.ap[0][1] * out_ap.ap[1][1] == num_idxs and num_idxs % 128 == 0
    assert in_ap.ap[0][0] == elem_step
    stride_bytes = elem_step * mybir.dt.size(in_ap.dtype)
    stride_bytes_256 = stride_bytes // 256
    assert stride_bytes_256 * 256 == stride_bytes and stride_bytes_256 < 256
    _in_ap = gp.lower_ap_dma(in_ap, for_custom_bir_dma=True)
    _idxs_ap = gp.lower_ap(idxs_ap)
    _out_ap = gp.lower_ap(out_ap)
    return gp.add_instruction(
        mybir.InstDMAGatherAnt(
            name=gp.bass.get_next_instruction_name(),
            ins=[*_in_ap, _idxs_ap, gp.lower_val_access(gp.to_reg(num_idxs))],
            outs=[_out_ap],
            transpose=False, num_idxs=num_idxs, elem_size=elem_size,
            stride_bytes_256=stride_bytes_256, gen_mode=0, single_packet=True,
            queue_num=queue_num, sbuf_tokens_per_rank=0,
            sbuf_free_dim_per_rank=0, sbuf_free_dim_pad_per_rank=0,
            sbuf_byte_offset=0))

def _build(Tbw_c, Tbw_v, b_score_val):
    """Build the shared SPMD program."""

    Tt_c, Tt_v = int(Tbw_c.sum()), int(Tbw_v.sum())
    Pk_c = int((-(-Tbw_c // TPP)).sum())
    Pk_v = int((-(-Tbw_v // TPP)).sum())

    nc = bacc.Bacc("TRN2", target_bir_lowering=False, num_devices=CORES,
                   num_swdge_queues=4)
    AluOp = mybir.AluOpType
    Act = mybir.ActivationFunctionType

    def ein(name, shape, dtype):
        return nc.dram_tensor(name, shape, dtype, kind="ExternalInput")

    vfT = ein("vfT", [VF, V_S], F32)
    cfT = ein("cfT", [CF, C_S], F32)
    wvar = ein("wvar", [VF, H], F32)
    wcon = ein("wcon", [CF, H], F32)
    wv2c = ein("wv2c", [H, H], F32)
    wc2v = ein("wc2v", [H, H], F32)
    wsco = ein("wsco", [H, 1], BF16)
    bvar = ein("bvar", [H, 1], F32)
    bcon = ein("bcon", [H, 1], F32)
    bv2c = ein("bv2c", [H, 1], F32)
    bc2v = ein("bc2v", [H, 1], F32)
    idx_v2c_d = ein("idx_v2c", [P, Tt_c * 8], I16)
    idx_c2v_d = ein("idx_c2v", [P, Tt_v * 8], I16)
    pdst_v2c_d = ein("pdst_v2c", [P, Pk_c], F32)
    pdst_c2v_d = ein("pdst_c2v", [P, Pk_v], F32)
    sfix_d = ein("sfix", [P, SLOTS], BF16)
    iota_d = ein("iota", [P, P], F32)
    ident_d = ein("ident", [H, H], BF16)
    scores_out = nc.dram_tensor("scores", [V_S], F32, kind="ExternalOutput")

    with tile.TileContext(nc) as tc:
        with (
            tc.tile_pool(name="const", bufs=1) as cpool,
            tc.tile_pool(name="state", bufs=1) as spool,
            tc.tile_pool(name="dram", bufs=1, space="DRAM") as dpool,
            tc.tile_pool(name="gpool", bufs=14) as gpool,
            tc.tile_pool(name="parts", bufs=3) as parts_pool,
            tc.tile_pool(name="s2p", bufs=3) as s2_pool,
            tc.tile_pool(name="rowp", bufs=2) as row_pool,
            tc.tile_pool(name="ps_pack", bufs=3, space="PSUM") as ps_pack,
            tc.tile_pool(name="ps_s2o", bufs=3, space="PSUM") as ps_s2o,
            tc.tile_pool(name="ps_misc", bufs=2, space="PSUM") as ps_misc,
        ):
            def load_const(name, dram, shape, dtype):
                t = cpool.tile(shape, dtype, name=name)
                nc.sync.dma_start(out=t[:], in_=dram[:])
                return t

            sfix_sb = load_const("sfix_sb", sfix_d, [P, SLOTS], BF16)
            iota_sb = load_const("iota_sb", iota_d, [P, P], F32)
            ident_sb = load_const("ident_sb", ident_d, [H, H], BF16)
            wvar_sb = load_const("wvar_sb", wvar, [VF, H], F32)
            wcon_sb = load_const("wcon_sb", wcon, [CF, H], F32)
            wv2c_sb = load_const("wv2c_sb", wv2c, [H, H], F32)
            wc2v_sb = load_const("wc2v_sb", wc2v, [H, H], F32)
            wsco_sb = load_const("wsco_sb", wsco, [H, 1], BF16)
            bvar_sb = load_const("bvar_sb", bvar, [H, 1], F32)
            bcon_sb = load_const("bcon_sb", bcon, [H, 1], F32)
            bv2c_sb = load_const("bv2c_sb", bv2c, [H, 1], F32)
            bc2v_sb = load_const("bc2v_sb", bc2v, [H, 1], F32)
            idx_sh = cpool.tile([P, max(Tt_c, Tt_v) * 8], I16, name="idx_sh")
            pdst_v2c_sb = load_const("pdst_v2c_sb", pdst_v2c_d, [P, Pk_c], F32)
            pdst_c2v_sb = load_const("pdst_c2v_sb", pdst_c2v_d, [P, Pk_v], F32)

            szero_sb = cpool.tile([P, SLOTS], BF16, name="szero_sb")
            nc.vector.memset(szero_sb[:], 0.0)
            gdummy_sb = cpool.tile([P, H], BF16, name="gdummy_sb")
            nc.vector.memset(gdummy_sb[:], 0.0)
            zrow_sb = cpool.tile([P, ROWB], BF16, name="zrow_sb")
            nc.vector.memset(zrow_sb[:], 0.0)

            hvT = spool.tile([H, V_S], BF16, name="hvT")
            hcT = spool.tile([H, C_S], BF16, name="hcT")
            macc_sh = spool.tile([H, NBU_V * P], F32, name="macc_sh")
            tabs_v = [dpool.tile([RV, ROWB], BF16, name=f"tab_v{i}",
                                 addr_space="Shared", tag=f"tab_v{i}")
                      for i in range(ROUNDS)]
            tabs_c = [dpool.tile([RC, ROWB], BF16, name=f"tab_c{i}",
                                 addr_space="Shared", tag=f"tab_c{i}")
                      for i in range(ROUNDS)]
            agin_v = dpool.tile([V_S, ROWB], BF16, name="agin_v")
            agin_c = dpool.tile([C_S, ROWB], BF16, name="agin_c")

            # zero the shard tail (dummy rows shipped by every AllGather)
            nc.sync.dma_start(out=agin_v[NBU_V * P:V_S, :], in_=zrow_sb[:])
            nc.sync.dma_start(out=agin_c[NBU_C * P:C_S, :], in_=zrow_sb[:])

            # ---- initial embeddings hT = relu(W.T @ featT + b)
            def emit_init(featT_dram, fdim, n_s, w_sb, b_sb, hT):
                with tc.tile_pool(name="initp", bufs=2) as ipool:
                    c0 = 0
                    while c0 < n_s:
                        w = min(512, n_s - c0)
                        fch = ipool.tile([fdim, 512], F32, name="fch", tag="fch")
                        nc.sync.dma_start(out=fch[:, :w],
                                          in_=featT_dram[:, c0:c0 + w])
                        psi = ps_misc.tile([H, 512], F32, name="psi", tag="misc")
                        nc.tensor.matmul(out=psi[:, :w], lhsT=w_sb[:],
                                         rhs=fch[:, :w], start=True, stop=True)
                        nc.scalar.activation(out=hT[:, c0:c0 + w], in_=psi[:, :w],
                                             func=Act.Relu, bias=b_sb[:])
                        c0 += w

            emit_init(vfT, VF, V_S, wvar_sb, bvar_sb, hvT)
            emit_init(cfT, CF, C_S, wcon_sb, bcon_sb, hcT)

            def emit_writeback(hT, nblk, agin, tab, nrows):
                rstage = row_pool.tile([P, NBU_V * H], BF16, name="rstage",
                                       tag="rstage")
                for b in range(nblk):
                    psr = ps_misc.tile([P, H], BF16, name="psr", tag="misc")
                    nc.tensor.transpose(out=psr[:], in_=hT[:, b * P:(b + 1) * P],
                                        identity=ident_sb[:])
                    nc.vector.tensor_copy(out=rstage[:, b * H:(b + 1) * H],
                                          in_=psr[:])
                nc.sync.dma_start(
                    out=agin[0:nblk * P, 0:H].rearrange("(b p) f -> p b f", p=P),
                    in_=rstage[:, :nblk * H].rearrange("p (b f) -> p b f", f=H))
                nc.gpsimd.collective_compute(
                    "AllGather", mybir.AluOpType.bypass,
                    replica_groups=[list(range(CORES))],
                    ins=[agin[:]], outs=[tab[:]])

            emit_writeback(hvT, NBU_V, agin_v, tabs_v[0], RV)

            # ---- one message-passing phase
            def emit_phase(tab_src, wrows, idx_dram, ncols, pdst_sb, Tbw, nblk,
                           macc, hT, W_sb, b_sb, writeback):
                nwin = Tbw.shape[0]
                idx_sb = idx_sh
                nc.sync.dma_start(out=idx_sb[:, :ncols], in_=idx_dram[:])
                nc.vector.memset(macc[:, :nblk * P], 0.0)
                g_tiles = {}

                def g_ap(w, base_w, tw):
                    cidx = tw // TPC
                    if (w, cidx) not in g_tiles:
                        ntile = min(TPC, int(Tbw[w].sum()) - cidx * TPC)
                        g = gpool.tile([P, TPC, H], BF16, name="g", tag="g")
                        _dma_gather_raw(
                            nc.gpsimd, g[:, :ntile, :],
                            tab_src[w * wrows:(w + 1) * wrows, 0:H],
                            idx_sb[:, (base_w + cidx * TPC) * 8:
                                   (base_w + cidx * TPC + ntile) * 8],
                            num_idxs=ntile * P, elem_size=H, elem_step=ROWB,
                            queue_num=cidx % 4)
                        g_tiles[(w, cidx)] = g
                    return g_tiles[(w, cidx)][:, tw % TPC, :]

                win_base = np.concatenate([[0], np.cumsum(Tbw.sum(1))])
                pk = 0
                for w in range(nwin):
                    tw = 0
                    for b in range(nblk):
                        npk = -(-int(Tbw[w, b]) // TPP)
                        for p_ in range(npk):
                            psp = ps_pack.tile([P, H], F32, name="psp", tag="psp")
                            for j in range(TPP):
                                t = TPP * p_ + j
                                if t < Tbw[w, b]:
                                    lhs = sfix_sb[:]
                                    rhs = g_ap(w, int(win_base[w]), tw + t)
                                else:
                                    lhs, rhs = szero_sb[:], gdummy_sb[:]
                                nc.tensor.matmul(
                                    out=psp[j * SLOTS:(j + 1) * SLOTS, :],
                                    lhsT=lhs, rhs=rhs, start=True, stop=True,
                                    tile_position=(0, j * SLOTS))
                            parts = parts_pool.tile([P, H], F32, name="parts",
                                                    tag="parts")
                            nc.vector.tensor_copy(out=parts[:], in_=psp[:])
                            s2 = s2_pool.tile([P, P], F32, name="s2", tag="s2")
                            nc.vector.tensor_tensor(
                                out=s2[:], in0=iota_sb[:],
                                in1=pdst_sb[:, pk:pk + 1].to_broadcast([P, P]),
                                op=AluOp.is_equal)
                            pso = ps_s2o.tile([H, P], F32, name="pso", tag="pso")
                            nc.tensor.matmul(out=pso[:], lhsT=parts[:], rhs=s2[:],
                                             start=True, stop=True)
                            nc.vector.tensor_tensor(
                                out=macc[:, b * P:(b + 1) * P],
                                in0=macc[:, b * P:(b + 1) * P], in1=pso[:],
                                op=AluOp.add)
                            pk += 1
                        tw += int(Tbw[w, b])

                for b in range(nblk):
                    psu = ps_misc.tile([H, P], F32, name="psu", tag="misc")
                    nc.tensor.matmul(out=psu[:], lhsT=W_sb[:],
                                     rhs=macc[:, b * P:(b + 1) * P],
                                     start=True, stop=True)
                    tmp = s2_pool.tile([H, P], F32, name="tmp", tag="tmp")
                    nc.vector.tensor_tensor(out=tmp[:], in0=psu[:],
                                            in1=hT[:, b * P:(b + 1) * P],
                                            op=AluOp.add)
                    nc.scalar.activation(out=hT[:, b * P:(b + 1) * P], in_=tmp[:],
                                         func=Act.Relu, bias=b_sb[:])
                if writeback is not None:
                    writeback()

            for r in range(ROUNDS):
                emit_phase(tabs_v[r], VWROWS, idx_v2c_d, Tt_c * 8, pdst_v2c_sb,
                           Tbw_c, NBU_C, macc_sh, hcT, wv2c_sb, bv2c_sb,
                           lambda r=r: emit_writeback(hcT, NBU_C, agin_c,
                                                      tabs_c[r], RC))
                last = r == ROUNDS - 1
                emit_phase(tabs_c[r], CWROWS, idx_c2v_d, Tt_v * 8, pdst_c2v_sb,
                           Tbw_v, NBU_V, macc_sh, hvT, wc2v_sb, bc2v_sb,
                           None if last else
                           (lambda r=r: emit_writeback(hvT, NBU_V, agin_v,
                                                       tabs_v[r + 1], RV)))

            # ---- scores = h_var @ w_score + b_score (shard)
            c0 = 0
            while c0 < V_S:
                w = min(512, V_S - c0)
                pss = ps_misc.tile([1, 512], F32, name="pss", tag="misc")
                nc.tensor.matmul(out=pss[:, :w], lhsT=wsco_sb[:],
                                 rhs=hvT[:, c0:c0 + w], start=True, stop=True)
                sch = s2_pool.tile([1, 512], F32, name="sch", tag="sch")
                nc.vector.tensor_scalar(
                    out=sch[:, :w], in0=pss[:, :w],
                    scalar1=float(b_score_val), scalar2=None, op0=AluOp.add)
                nc.sync.dma_start(out=scores_out[None, c0:c0 + w],
                                  in_=sch[0:1, :w])
                c0 += w

    nc.compile()
    return nc


_CACHE = {}


def kernel(**inputs):
    var_feat = np.asarray(inputs["var_feat"], np.float32)
    constr_feat = np.asarray(inputs["constr_feat"], np.float32)
    var_idx = np.asarray(inputs["var_idx"]).astype(np.int64)
    constr_idx = np.asarray(inputs["constr_idx"]).astype(np.int64)
    b_score_val = float(np.asarray(inputs["b_score"]).reshape(-1)[0])

    key = (var_idx.tobytes(), constr_idx.tobytes())
    if key in _CACHE:
        nc, idx_v, pdst_v, idx_c, pdst_c = _CACHE[key]
    else:
        # v2c: dst=constr, src=var
        idx_v, pdst_v, Tbw_c = _prep_direction(
            constr_idx, var_idx, C_CORE, NBU_C, VWIN, V_CORE, V_S, VDUMMY)
        # c2v: dst=var, src=constr
        idx_c, pdst_c, Tbw_v = _prep_direction(
            var_idx, constr_idx, V_CORE, NBU_V, CWIN, C_CORE, C_S, CDUMMY)
        nc = _build(Tbw_c, Tbw_v, b_score_val)
        _CACHE[key] = (nc, idx_v, pdst_v, idx_c, pdst_c)

    iota = np.broadcast_to(np.arange(P, dtype=np.float32), (P, P)).copy()
    sfix = (np.arange(P)[:, None] // WSLOT == np.arange(SLOTS)[None, :]).astype(BF)
    ident = np.eye(H, dtype=np.float32).astype(BF)

    vf_pad = np.zeros((CORES, V_S, VF), np.float32)
    vf_pad[:, :V_CORE] = var_feat.reshape(CORES, V_CORE, VF)
    cf_pad = np.zeros((CORES, C_S, CF), np.float32)
    cf_pad[:, :C_CORE] = constr_feat.reshape(CORES, C_CORE, CF)

    common = dict(
        wvar=np.ascontiguousarray(inputs["W_var"], dtype=np.float32),
        wcon=np.ascontiguousarray(inputs["W_con"], dtype=np.float32),
        wv2c=np.ascontiguousarray(inputs["W_v2c"], dtype=np.float32),
        wc2v=np.ascontiguousarray(inputs["W_c2v"], dtype=np.float32),
        wsco=np.ascontiguousarray(inputs["W_score"], dtype=np.float32).astype(BF),
        bvar=np.ascontiguousarray(inputs["b_var"], dtype=np.float32).reshape(H, 1),
        bcon=np.ascontiguousarray(inputs["b_con"], dtype=np.float32).reshape(H, 1),
        bv2c=np.ascontiguousarray(inputs["b_v2c"], dtype=np.float32).reshape(H, 1),
        bc2v=np.ascontiguousarray(inputs["b_c2v"], dtype=np.float32).reshape(H, 1),
        sfix=sfix, iota=iota, ident=ident,
    )
    in_maps = []
    for k in range(CORES):
        m = dict(common)
        m["vfT"] = np.ascontiguousarray(vf_pad[k].T)
        m["cfT"] = np.ascontiguousarray(cf_pad[k].T)
        m["idx_v2c"] = idx_v[k]
        m["pdst_v2c"] = pdst_v[k]
        m["idx_c2v"] = idx_c[k]
        m["pdst_c2v"] = pdst_c[k]
        in_maps.append(m)

    res = run_bass_kernel_spmd(nc, in_maps, list(range(CORES)))
    scores = np.concatenate([res.results[k]["scores"].reshape(-1)[:V_CORE]
                             for k in range(CORES)])
    return scores.astype(np.float32)



# revision 3
# speedup vs baseline: 3.3827x; 3.3827x over previous
"""Trainium2 Bass kernel: bipartite GNN message passing (BranchingGNN), 8-core SPMD.

Sharding: core k owns constraint rows [k*6250,(k+1)*6250) and variable rows
[k*12500,(k+1)*12500); each core processes all edges targeting its shard, so
messages need no cross-core reduction. Node tables are compact row-major
[N, 64] bf16 in DRAM and are re-broadcast each phase by an AllGather of the
updated shard.

Per phase (one message direction):
  - edges sorted by (src-window, src-parity, dst-block, src-row); each
    (window,parity,block) group is padded to 128-edge tiles. Source rows are
    gathered by dma_gather (128B rows at 256B stride over the even/odd row
    subsequence of the compact table), ascending addresses for HBM locality.
  - per tile, a one-hot S [128e,128d] = (iota == dstcol) is built on DVE
    (batched per gather call); one PE matmul per tile accumulates
    msgT [64,128] into a per-group PSUM tile (start/stop over the group).
  - group partials are added into an SBUF accumulator macc per dst block;
    the block update relu(hT + W.T @ msgT + b) runs as soon as its last
    group lands; PE transpose + one DMA + AllGather republish the table.
"""
import sys

sys.path.insert(0, "/opt/trn_rl_repo")

import numpy as np
import ml_dtypes

import concourse.bass as bass
import concourse.bacc as bacc
import concourse.mybir as mybir
import concourse.tile as tile
from concourse.bass_utils import run_bass_kernel_spmd

# ---- problem constants
V, C, E = 100000, 50000, 1250000
VF, CF, H = 32, 32, 64
ROUNDS = 3
CORES = 8
P = 128
TPC = 7               # tiles per gather call (ring cap 64 descs)

V_CORE, C_CORE = 12500, 6250          # real nodes per core
V_S, C_S = 12672, 6400                # shard rows (99 / 50 blocks)
NB_V, NB_C = 98, 49                   # updated dst blocks per core
RV, RC = CORES * V_S, CORES * C_S     # 101376 / 51200 table rows
VWIN, CWIN = 2, 1                     # source windows (half-row reach 32767)

BF16 = mybir.dt.bfloat16
F32 = mybir.dt.float32
I16 = mybir.dt.int16
BF = ml_dtypes.bfloat16


def _prep_direction(dst, src, n_dst_core, nblk, nwin, n_src_core, src_s):
    """Edge metadata for one direction.

    Returns (idx16 [CORES,128,Ttot*8], pdst [CORES,128,Ttot] bf16,
             T_g [NG] tiles per (w,h,b) group).
    """
    dst = np.asarray(dst, np.int64)
    src = np.asarray(src, np.int64)

    row = (src // n_src_core) * src_s + src % n_src_core
    wsize = CORES * src_s // nwin
    w = row // wsize
    h = row % 2
    half = (row % wsize) // 2

    core = dst // n_dst_core
    dloc = dst % n_dst_core
    b = dloc // P
    j = dloc % P

    NG = nwin * 2 * nblk
    grp = (w * 2 + h) * nblk + b

    order = np.lexsort((half, grp, core))
    grp_s = grp[order]
    core_s = core[order]
    half_s = half[order]
    j_s = j[order]

    cnt = np.bincount(core * NG + grp, minlength=CORES * NG).reshape(CORES, NG)
    T_g = np.maximum(-(-cnt.max(0) // P), (cnt.max(0) > 0).astype(np.int64))
    TB = np.cumsum(T_g) - T_g
    Ttot = int(T_g.sum())

    # rank within (core, grp)
    run_start = np.zeros(CORES * NG + 1, np.int64)
    run_start[1:] = np.cumsum(cnt.reshape(-1))
    keyf = core_s * NG + grp_s
    rank = np.arange(dst.size, dtype=np.int64) - run_start[keyf]
    slot = TB[grp_s] * P + rank

    idx16 = np.zeros((CORES, Ttot * P), np.int16)
    valid = np.zeros((CORES, Ttot * P), bool)
    idx16[core_s, slot] = half_s.astype(np.int16)
    valid[core_s, slot] = True
    pdst = np.full((CORES, Ttot * P), -1.0, np.float32)
    pdst[core_s, slot] = j_s

    # forward-fill pad idx with the previous valid half (address locality)
    for k in range(CORES):
        v = valid[k]
        pos = np.where(v, np.arange(Ttot * P), 0)
        np.maximum.accumulate(pos, out=pos)
        idx16[k] = idx16[k][pos]

    # wrap layout for dma_gather: [128, Ttot*8]
    packed = np.zeros((CORES, P, Ttot * 8), np.int16)
    for k in range(CORES):
        a = idx16[k].reshape(-1, 16).T
        packed[k] = np.tile(a, (8, 1))

    pdst_t = pdst.reshape(CORES, Ttot, P).transpose(0, 2, 1)  # [CORES,128,Ttot]
    return packed, np.ascontiguousarray(pdst_t).astype(BF), T_g.astype(int)


def _plan(T_g, nblk, nwin):
    """Compile-time schedule: per-call and per-tile metadata."""
    NG = len(T_g)
    calls = []          # (w, h, t0, ntile, grp_list per tile)
    tiles = []          # (grp, first_in_grp, last_in_grp)
    TB = np.cumsum(T_g) - T_g
    for wh in range(nwin * 2):
        g0, g1 = wh * nblk, (wh + 1) * nblk
        t0 = int(TB[g0])
        t1 = int(TB[g1 - 1] + T_g[g1 - 1])
        t = t0
        while t < t1:
            nt = min(TPC, t1 - t)
            calls.append((wh // 2, wh % 2, t, nt))
            t += nt
    for g in range(NG):
        for i in range(int(T_g[g])):
            tiles.append((g, i == 0, i == int(T_g[g]) - 1))
    # per block: ordered groups with T>0
    blk_groups = [[] for _ in range(nblk)]
    for g in range(NG):
        if T_g[g] > 0:
            blk_groups[g % nblk].append(g)
    return calls, tiles, blk_groups


def _dma_gather_raw(gp, out_ap, in_ap, idxs_ap, num_idxs, elem_size, elem_step,
                    queue_num=0):
    """dma_gather (non-transpose, HBM source) allowing 128B rows at 256B stride."""
    from concourse import ap_utils
    gp._assert_queue_num(queue_num)
    assert idxs_ap.dtype == mybir.dt.int16
    assert in_ap.dtype == out_ap.dtype
    assert ap_utils.ap_is_contiguous(in_ap.ap[1:])
    assert ap_utils.ap_is_contiguous(out_ap.ap[1:])
    assert ap_utils.ap_is_contiguous(idxs_ap.ap[1:])
    assert in_ap.ap[-1][1] == out_ap.ap[-1][1] == elem_size
    assert out_ap.ap[0][1] * out_ap.ap[1][1] == num_idxs and num_idxs % 128 == 0
    assert in_ap.ap[0][0] == elem_step
    stride_bytes = elem_step * mybir.dt.size(in_ap.dtype)
    stride_bytes_256 = stride_bytes // 256
    assert stride_bytes_256 * 256 == stride_bytes and stride_bytes_256 < 256
    _in_ap = gp.lower_ap_dma(in_ap, for_custom_bir_dma=True)
    _idxs_ap = gp.lower_ap(idxs_ap)
    _out_ap = gp.lower_ap(out_ap)
    return gp.add_instruction(
        mybir.InstDMAGatherAnt(
            name=gp.bass.get_next_instruction_name(),
            ins=[*_in_ap, _idxs_ap, gp.lower_val_access(gp.to_reg(num_idxs))],
            outs=[_out_ap],
            transpose=False, num_idxs=num_idxs, elem_size=elem_size,
            stride_bytes_256=stride_bytes_256, gen_mode=0, single_packet=True,
            queue_num=queue_num, sbuf_tokens_per_rank=0,
            sbuf_free_dim_per_rank=0, sbuf_free_dim_pad_per_rank=0,
            sbuf_byte_offset=0))


def _build(T_gc, T_gv, b_score_val):
    Tt_c, Tt_v = int(T_gc.sum()), int(T_gv.sum())
    calls_c, tiles_c, bg_c = _plan(T_gc, NB_C, VWIN)
    calls_v, tiles_v, bg_v = _plan(T_gv, NB_V, CWIN)

    nc = bacc.Bacc("TRN2", target_bir_lowering=False, num_devices=CORES,
                   num_swdge_queues=4)
    AluOp = mybir.AluOpType
    Act = mybir.ActivationFunctionType

    def ein(name, shape, dtype):
        return nc.dram_tensor(name, shape, dtype, kind="ExternalInput")

    vfT = ein("vfT", [VF, V_S], F32)
    cfT = ein("cfT", [CF, C_S], F32)
    wvar = ein("wvar", [VF, H], F32)
    wcon = ein("wcon", [CF, H], F32)
    wv2c = ein("wv2c", [H, H], F32)
    wc2v = ein("wc2v", [H, H], F32)
    wsco = ein("wsco", [H, 1], BF16)
    bvar = ein("bvar", [H, 1], F32)
    bcon = ein("bcon", [H, 1], F32)
    bv2c = ein("bv2c", [H, 1], F32)
    bc2v = ein("bc2v", [H, 1], F32)
    idx_v2c_d = ein("idx_v2c", [P, Tt_c * 8], I16)
    idx_c2v_d = ein("idx_c2v", [P, Tt_v * 8], I16)
    pdst_v2c_d = ein("pdst_v2c", [P, Tt_c], BF16)
    pdst_c2v_d = ein("pdst_c2v", [P, Tt_v], BF16)
    iota_d = ein("iota", [P, P], BF16)
    ident_d = ein("ident", [H, H], BF16)
    scores_out = nc.dram_tensor("scores", [V_S], F32, kind="ExternalOutput")

    with tile.TileContext(nc) as tc:
        with (
            tc.tile_pool(name="const", bufs=1) as cpool,
            tc.tile_pool(name="state", bufs=1) as spool,
            tc.tile_pool(name="dram", bufs=1, space="DRAM") as dpool,
            tc.tile_pool(name="gpool", bufs=12) as gpool,
            tc.tile_pool(name="s_pool", bufs=6) as s_pool,
            tc.tile_pool(name="misc", bufs=4) as mpool,
            tc.tile_pool(name="ps_acc", bufs=4, space="PSUM") as ps_acc,
            tc.tile_pool(name="ps_upd", bufs=2, space="PSUM") as ps_upd,
            tc.tile_pool(name="ps_misc", bufs=2, space="PSUM") as ps_misc,
        ):
            def load_const(name, dram, shape, dtype):
                t = cpool.tile(shape, dtype, name=name)
                nc.sync.dma_start(out=t[:], in_=dram[:])
                return t

            iota_sb = load_const("iota_sb", iota_d, [P, P], BF16)
            ident_sb = load_const("ident_sb", ident_d, [H, H], BF16)
            wvar_sb = load_const("wvar_sb", wvar, [VF, H], F32)
            wcon_sb = load_const("wcon_sb", wcon, [CF, H], F32)
            wv2c_sb = load_const("wv2c_sb", wv2c, [H, H], F32)
            wc2v_sb = load_const("wc2v_sb", wc2v, [H, H], F32)
            wsco_sb = load_const("wsco_sb", wsco, [H, 1], BF16)
            bvar_sb = load_const("bvar_sb", bvar, [H, 1], F32)
            bcon_sb = load_const("bcon_sb", bcon, [H, 1], F32)
            bv2c_sb = load_const("bv2c_sb", bv2c, [H, 1], F32)
            bc2v_sb = load_const("bc2v_sb", bc2v, [H, 1], F32)
            idx_c_sb = load_const("idx_c_sb", idx_v2c_d, [P, Tt_c * 8], I16)
            idx_v_sb = load_const("idx_v_sb", idx_c2v_d, [P, Tt_v * 8], I16)
            pdst_c_sb = load_const("pdst_c_sb", pdst_v2c_d, [P, Tt_c], BF16)
            pdst_v_sb = load_const("pdst_v_sb", pdst_c2v_d, [P, Tt_v], BF16)

            hvT = spool.tile([H, V_S], BF16, name="hvT")
            hcT = spool.tile([H, C_S], BF16, name="hcT")
            macc = spool.tile([H, NB_V * P], F32, name="macc")
            rstage = spool.tile([P, NB_V * H], BF16, name="rstage")
            zrow_sb = cpool.tile([P, H], BF16, name="zrow_sb")
            nc.vector.memset(zrow_sb[:], 0.0)

            tabs_v = [dpool.tile([RV, H], BF16, name=f"tab_v{i}",
                                 addr_space="Shared", tag=f"tab_v{i}")
                      for i in range(ROUNDS)]
            tabs_c = [dpool.tile([RC, H], BF16, name=f"tab_c{i}",
                                 addr_space="Shared", tag=f"tab_c{i}")
                      for i in range(ROUNDS)]
            agin_v = dpool.tile([V_S, H], BF16, name="agin_v")
            agin_c = dpool.tile([C_S, H], BF16, name="agin_c")
            nc.sync.dma_start(out=agin_v[NB_V * P:V_S, :], in_=zrow_sb[:])
            nc.sync.dma_start(out=agin_c[NB_C * P:C_S, :], in_=zrow_sb[:])

            # ---- initial embeddings hT = relu(W.T @ featT + b)
            def emit_init(featT_dram, fdim, n_s, w_sb, b_sb, hT):
                with tc.tile_pool(name="initp", bufs=2) as ipool:
                    c0 = 0
                    while c0 < n_s:
                        wd = min(512, n_s - c0)
                        fch = ipool.tile([fdim, 512], F32, name="fch", tag="fch")
                        nc.sync.dma_start(out=fch[:, :wd],
                                          in_=featT_dram[:, c0:c0 + wd])
                        psi = ps_misc.tile([H, 512], F32, name="psi", tag="misc")
                        nc.tensor.matmul(out=psi[:, :wd], lhsT=w_sb[:],
                                         rhs=fch[:, :wd], start=True, stop=True)
                        nc.scalar.activation(out=hT[:, c0:c0 + wd], in_=psi[:, :wd],
                                             func=Act.Relu, bias=b_sb[:])
                        c0 += wd

            emit_init(vfT, VF, V_S, wvar_sb, bvar_sb, hvT)
            emit_init(cfT, CF, C_S, wcon_sb, bcon_sb, hcT)

            def emit_writeback(hT, nblk, agin, tab):
                for b in range(nblk):
                    psr = ps_misc.tile([P, H], BF16, name="psr", tag="misc")
                    nc.tensor.transpose(out=psr[:], in_=hT[:, b * P:(b + 1) * P],
                                        identity=ident_sb[:])
                    nc.vector.tensor_copy(out=rstage[:, b * H:(b + 1) * H],
                                          in_=psr[:])
                nc.sync.dma_start(
                    out=agin[0:nblk * P, :].rearrange("(b p) f -> p b f", p=P),
                    in_=rstage[:, :nblk * H].rearrange("p (b f) -> p b f", f=H))
                nc.gpsimd.collective_compute(
                    "AllGather", mybir.AluOpType.bypass,
                    replica_groups=[list(range(CORES))],
                    ins=[agin[:]], outs=[tab[:]])

            emit_writeback(hvT, NB_V, agin_v, tabs_v[0])

            # ---- one message-passing phase
            def emit_phase(tab_src, nrows_tab, nwin, idx_sb, pdst_sb, T_g,
                           calls, tiles, blk_groups, nblk, hT, W_sb, b_sb,
                           writeback):
                # even/odd row views of the compact table, per window
                tab2 = tab_src[:].rearrange("(n two) f -> n (two f)", two=2)
                whalf = (nrows_tab // nwin) // 2
                win_ap = {}
                for w in range(nwin):
                    for h in range(2):
                        win_ap[(w, h)] = tab2[w * whalf:(w + 1) * whalf,
                                              h * H:(h + 1) * H]

                TB = np.cumsum(T_g) - T_g
                accs = {}
                done_groups = [0] * nblk
                ti = 0

                def emit_update(b):
                    ps2 = ps_upd.tile([H, P], F32, name="ps2", tag="ps2")
                    nc.tensor.matmul(out=ps2[:], lhsT=W_sb[:],
                                     rhs=macc[:, b * P:(b + 1) * P],
                                     start=True, stop=True)
                    tmp = mpool.tile([H, P], F32, name="tmp", tag="tmp")
                    nc.vector.tensor_tensor(out=tmp[:], in0=ps2[:],
                                            in1=hT[:, b * P:(b + 1) * P],
                                            op=AluOp.add)
                    nc.scalar.activation(out=hT[:, b * P:(b + 1) * P],
                                         in_=tmp[:], func=Act.Relu, bias=b_sb[:])

                for ci, (w, h, t0, nt) in enumerate(calls):
                    g = gpool.tile([P, TPC, H], BF16, name="g", tag="g")
                    _dma_gather_raw(
                        nc.gpsimd, g[:, :nt, :], win_ap[(w, h)],
                        idx_sb[:, t0 * 8:(t0 + nt) * 8],
                        num_idxs=nt * P, elem_size=H, elem_step=2 * H,
                        queue_num=ci % 4)
                    S = s_pool.tile([P, TPC, P], BF16, name="S", tag="S")
                    nc.vector.tensor_tensor(
                        out=S[:, :nt, :],
                        in0=iota_sb[:, None, :].to_broadcast([P, nt, P]),
                        in1=pdst_sb[:, t0:t0 + nt, None].to_broadcast([P, nt, P]),
                        op=AluOp.is_equal)
                    for i in range(nt):
                        t = t0 + i
                        grp, first, last = tiles[t]
                        if first:
                            accs[grp] = ps_acc.tile([H, P], F32, name="acc",
                                                    tag="acc")
                        nc.tensor.matmul(out=accs[grp][:], lhsT=g[:, i, :],
                                         rhs=S[:, i, :], start=first, stop=last)
                        if last:
                            b = grp % nblk
                            glist = blk_groups[b]
                            if done_groups[b] == 0:
                                nc.vector.tensor_copy(
                                    out=macc[:, b * P:(b + 1) * P],
                                    in_=accs[grp][:])
                            else:
                                nc.vector.tensor_tensor(
                                    out=macc[:, b * P:(b + 1) * P],
                                    in0=macc[:, b * P:(b + 1) * P],
                                    in1=accs[grp][:], op=AluOp.add)
                            del accs[grp]
                            done_groups[b] += 1
                            if done_groups[b] == len(glist):
                                emit_update(b)
                if writeback is not None:
                    writeback()

            for r in range(ROUNDS):
                emit_phase(tabs_v[r], RV, VWIN, idx_c_sb, pdst_c_sb, T_gc,
                           calls_c, tiles_c, bg_c, NB_C, hcT, wv2c_sb, bv2c_sb,
                           lambda r=r: emit_writeback(hcT, NB_C, agin_c,
                                                      tabs_c[r]))
                last = r == ROUNDS - 1
                emit_phase(tabs_c[r], RC, CWIN, idx_v_sb, pdst_v_sb, T_gv,
                           calls_v, tiles_v, bg_v, NB_V, hvT, wc2v_sb, bc2v_sb,
                           None if last else
                           (lambda r=r: emit_writeback(hvT, NB_V, agin_v,
                                                       tabs_v[r + 1])))

            # ---- scores = h_var @ w_score + b_score (shard)
            c0 = 0
            while c0 < V_S:
                wd = min(512, V_S - c0)
                pss = ps_misc.tile([1, 512], F32, name="pss", tag="misc")
                nc.tensor.matmul(out=pss[:, :wd], lhsT=wsco_sb[:],
                                 rhs=hvT[:, c0:c0 + wd], start=True, stop=True)
                sch = mpool.tile([1, 512], F32, name="sch", tag="sch")
                nc.vector.tensor_scalar(
                    out=sch[:, :wd], in0=pss[:, :wd],
                    scalar1=float(b_score_val), scalar2=None, op0=AluOp.add)
                nc.sync.dma_start(out=scores_out[None, c0:c0 + wd],
                                  in_=sch[0:1, :wd])
                c0 += wd

    nc.compile()
    return nc


_CACHE = {}


def kernel(**inputs):
    var_feat = np.asarray(inputs["var_feat"], np.float32)
    constr_feat = np.asarray(inputs["constr_feat"], np.float32)
    var_idx = np.asarray(inputs["var_idx"]).astype(np.int64)
    constr_idx = np.asarray(inputs["constr_idx"]).astype(np.int64)
    b_score_val = float(np.asarray(inputs["b_score"]).reshape(-1)[0])

    key = (var_idx.tobytes(), constr_idx.tobytes())
    if key in _CACHE:
        nc, idx_v, pdst_v, idx_c, pdst_c = _CACHE[key]
    else:
        # v2c: dst=constr, src=var
        idx_v, pdst_v, T_gc = _prep_direction(
            constr_idx, var_idx, C_CORE, NB_C, VWIN, V_CORE, V_S)
        # c2v: dst=var, src=constr
        idx_c, pdst_c, T_gv = _prep_direction(
            var_idx, constr_idx, V_CORE, NB_V, CWIN, C_CORE, C_S)
        nc = _build(T_gc, T_gv, b_score_val)
        _CACHE[key] = (nc, idx_v, pdst_v, idx_c, pdst_c)

    iota = np.broadcast_to(np.arange(P, dtype=np.float32),
                           (P, P)).astype(BF).copy()
    ident = np.eye(H, dtype=np.float32).astype(BF)

    vf_pad = np.zeros((CORES, V_S, VF), np.float32)
    vf_pad[:, :V_CORE] = var_feat.reshape(CORES, V_CORE, VF)
    cf_pad = np.zeros((CORES, C_S, CF), np.float32)
    cf_pad[:, :C_CORE] = constr_feat.reshape(CORES, C_CORE, CF)

    common = dict(
        wvar=np.ascontiguousarray(inputs["W_var"], dtype=np.float32),
        wcon=np.ascontiguousarray(inputs["W_con"], dtype=np.float32),
        wv2c=np.ascontiguousarray(inputs["W_v2c"], dtype=np.float32),
        wc2v=np.ascontiguousarray(inputs["W_c2v"], dtype=np.float32),
        wsco=np.ascontiguousarray(inputs["W_score"], dtype=np.float32).astype(BF),
        bvar=np.ascontiguousarray(inputs["b_var"], dtype=np.float32).reshape(H, 1),
        bcon=np.ascontiguousarray(inputs["b_con"], dtype=np.float32).reshape(H, 1),
        bv2c=np.ascontiguousarray(inputs["b_v2c"], dtype=np.float32).reshape(H, 1),
        bc2v=np.ascontiguousarray(inputs["b_c2v"], dtype=np.float32).reshape(H, 1),
        iota=iota, ident=ident,
    )
    in_maps = []
    for k in range(CORES):
        m = dict(common)
        m["vfT"] = np.ascontiguousarray(vf_pad[k].T)
        m["cfT"] = np.ascontiguousarray(cf_pad[k].T)
        m["idx_v2c"] = idx_v[k]
        m["pdst_v2c"] = pdst_v[k]
        m["idx_c2v"] = idx_c[k]
        m["pdst_c2v"] = pdst_c[k]
        in_maps.append(m)

    res = run_bass_kernel_spmd(nc, in_maps, list(range(CORES)))
    scores = np.concatenate([res.results[k]["scores"].reshape(-1)[:V_CORE]
                             for k in range(CORES)])
    return scores.astype(np.float32)


# revision 10
# speedup vs baseline: 3.3831x; 1.0001x over previous
"""Trainium2 Bass kernel: bipartite GNN message passing (BranchingGNN), 8-core SPMD.

Sharding: core k owns constraint rows [k*6250,(k+1)*6250) and variable rows
[k*12500,(k+1)*12500); each core processes all edges targeting its shard, so
messages need no cross-core reduction. Node tables are compact row-major
[N, 64] bf16 in DRAM and are re-broadcast each phase by an AllGather of the
updated shard.

Per phase (one message direction):
  - edges sorted by (src-window, src-parity, dst-block, src-row); each
    (window,parity,block) group is padded to 128-edge tiles. Source rows are
    gathered by dma_gather (128B rows at 256B stride over the even/odd row
    subsequence of the compact table), ascending addresses for HBM locality.
  - per tile, a one-hot S [128e,128d] = (iota == dstcol) is built on DVE
    (batched per gather call); one PE matmul per tile accumulates
    msgT [64,128] into a per-group PSUM tile (start/stop over the group).
  - group partials are added into an SBUF accumulator macc per dst block;
    the block update relu(hT + W.T @ msgT + b) runs as soon as its last
    group lands; PE transpose + one DMA + AllGather republish the table.
"""
import sys

sys.path.insert(0, "/opt/trn_rl_repo")

import numpy as np
import ml_dtypes

import concourse.bass as bass
import concourse.bacc as bacc
import concourse.mybir as mybir
import concourse.tile as tile
from concourse.bass_utils import run_bass_kernel_spmd

# ---- problem constants
V, C, E = 100000, 50000, 1250000
VF, CF, H = 32, 32, 64
ROUNDS = 3
CORES = 8
P = 128
TPC = 7               # tiles per gather call (ring cap 64 descs)

V_CORE, C_CORE = 12500, 6250          # real nodes per core
V_S, C_S = 12672, 6400                # shard rows (99 / 50 blocks)
NB_V, NB_C = 98, 49                   # updated dst blocks per core
RV, RC = CORES * V_S, CORES * C_S     # 101376 / 51200 table rows
VWIN, CWIN = 2, 1                     # source windows (half-row reach 32767)

BF16 = mybir.dt.bfloat16
F32 = mybir.dt.float32
I16 = mybir.dt.int16
BF = ml_dtypes.bfloat16


def _prep_direction(dst, src, n_dst_core, nblk, nwin, n_src_core, src_s):
    """Edge metadata for one direction, pair-slot layout.

    Per (core, w, h, b) group: edges of each dst j are paired; full pairs
    (up to a min-over-cores pack budget) form 2-tile packs whose slots are
    pair-sums; leftovers go to raw 1-tile units (slot per edge). Returns
    (idx16 [CORES,128,Ttot*8], pdst [CORES,128,MUtot] bf16,
     npk2 [NG], nraw [NG]).
    """
    dst = np.asarray(dst, np.int64)
    src = np.asarray(src, np.int64)
    NE = dst.size

    row = (src // n_src_core) * src_s + src % n_src_core
    wsize = CORES * src_s // nwin
    w = row // wsize
    h = row % 2
    half = (row % wsize) // 2

    core = dst // n_dst_core
    dloc = dst % n_dst_core
    b = dloc // P
    j = dloc % P

    NG = nwin * 2 * nblk
    grp = (w * 2 + h) * nblk + b

    order = np.lexsort((half, j, grp, core))
    grp_s, core_s, half_s, j_s = grp[order], core[order], half[order], j[order]

    # rank within (core, grp, j) run
    rid = (core_s * NG + grp_s) * P + j_s
    rcnt = np.bincount(rid, minlength=CORES * NG * P)
    rstart = np.zeros(rcnt.size + 1, np.int64)
    rstart[1:] = np.cumsum(rcnt)
    rank = np.arange(NE, dtype=np.int64) - rstart[rid]
    nrun = rcnt[rid]
    is_pair = rank < (nrun - nrun % 2)

    # pair-slot number within (core, grp): pairs of earlier j-runs + own
    pj = rcnt // 2                                    # pairs per run
    pj_cg = pj.reshape(CORES * NG, P)
    pj_cum = np.cumsum(pj_cg, 1) - pj_cg              # pairs before run, in-grp
    pair_slot = pj_cum.reshape(-1)[rid] + rank // 2   # valid where is_pair
    elem = rank % 2

    S2 = pj_cg.sum(1).reshape(CORES, NG)              # pair slots per core/grp
    npk2 = (S2 // P).min(0)                           # full packs per grp
    in_pack = is_pair & (pair_slot < npk2[grp_s] * P)

    # raw-slot rank per (core, grp) among non-packed edges, keep sort order
    kcg = core_s * NG + grp_s
    raw_mask = ~in_pack
    raw_rank = np.zeros(NE, np.int64)
    kraw = kcg[raw_mask]
    o2 = np.argsort(kraw, kind="stable")
    cnt_raw = np.bincount(kraw, minlength=CORES * NG)
    st = np.zeros(CORES * NG + 1, np.int64)
    st[1:] = np.cumsum(cnt_raw)
    rr = np.empty(kraw.size, np.int64)
    rr[o2] = np.arange(kraw.size) - st[kraw[o2]]
    raw_rank[raw_mask] = rr
    nraw = -(-cnt_raw.reshape(CORES, NG).max(0) // P)

    T_g = 2 * npk2 + nraw                             # tiles per grp
    MU_g = npk2 + nraw                                # matmul units per grp
    TB = np.cumsum(T_g) - T_g
    MB = np.cumsum(MU_g) - MU_g
    Ttot, MUtot = int(T_g.sum()), int(MU_g.sum())

    # flat idx position and (mu, slot) per edge
    pk = pair_slot // P
    ps = pair_slot % P
    tpos = np.where(in_pack,
                    (TB[grp_s] + 2 * pk + elem) * P + ps,
                    (TB[grp_s] + 2 * npk2[grp_s] + raw_rank // P) * P
                    + raw_rank % P)
    mu = np.where(in_pack, MB[grp_s] + pk,
                  MB[grp_s] + npk2[grp_s] + raw_rank // P)
    mslot = np.where(in_pack, ps, raw_rank % P)

    idx16 = np.zeros((CORES, Ttot * P), np.int16)
    valid = np.zeros((CORES, Ttot * P), bool)
    idx16[core_s, tpos] = half_s.astype(np.int16)
    valid[core_s, tpos] = True
    pdst = np.full((CORES, MUtot * P), -1.0, np.float32)
    pdst[core_s, mu * P + mslot] = j_s

    for k in range(CORES):
        v = valid[k]
        pos = np.where(v, np.arange(Ttot * P), 0)
        np.maximum.accumulate(pos, out=pos)
        idx16[k] = idx16[k][pos]

    packed = np.zeros((CORES, P, Ttot * 8), np.int16)
    for k in range(CORES):
        a = idx16[k].reshape(-1, 16).T
        packed[k] = np.tile(a, (8, 1))

    pdst_t = pdst.reshape(CORES, MUtot, P).transpose(0, 2, 1)
    return packed, np.ascontiguousarray(pdst_t).astype(BF), npk2, nraw


def _plan(npk2, nraw, nblk, nwin):
    """Compile-time schedule.

    Returns (calls, units, blk_groups): calls = (w, h, t0, mu0, unit_idx
    list); units[u] = (grp, is_pack, tile_off_in_grp, first_mu, last_mu).
    """
    NG = len(npk2)
    T_g = 2 * npk2 + nraw
    MU_g = npk2 + nraw
    TB = np.cumsum(T_g) - T_g
    MB = np.cumsum(MU_g) - MU_g
    units = []
    for g in range(NG):
        for k in range(int(npk2[g])):
            units.append((g, True, 2 * k, k == 0,
                          k == int(MU_g[g]) - 1))
        for r in range(int(nraw[g])):
            units.append((g, False, 2 * int(npk2[g]) + r,
                          int(npk2[g]) + r == 0,
                          int(npk2[g]) + r == int(MU_g[g]) - 1))
    calls = []
    for wh in range(nwin * 2):
        g0, g1 = wh * nblk, (wh + 1) * nblk
        u = int(MB[g0])
        u_end = int(MB[g1 - 1] + MU_g[g1 - 1])
        while u < u_end:
            nt = 0
            ulist = []
            while u < u_end and nt + (2 if units[u][1] else 1) <= TPC:
                ulist.append(u)
                nt += 2 if units[u][1] else 1
                u += 1
            g0t, _, off0, _, _ = units[ulist[0]]
            t0 = int(TB[g0t]) + off0
            calls.append((wh // 2, wh % 2, t0, ulist[0], ulist))
    blk_groups = [[] for _ in range(nblk)]
    for g in range(NG):
        if MU_g[g] > 0:
            blk_groups[g % nblk].append(g)
    return calls, units, blk_groups


def _dma_gather_raw(gp, out_ap, in_ap, idxs_ap, num_idxs, elem_size, elem_step,
                    queue_num=0):
    """dma_gather (non-transpose, HBM source) allowing 128B rows at 256B stride."""
    from concourse import ap_utils
    gp._assert_queue_num(queue_num)
    assert idxs_ap.dtype == mybir.dt.int16
    assert in_ap.dtype == out_ap.dtype
    assert ap_utils.ap_is_contiguous(in_ap.ap[1:])
    assert ap_utils.ap_is_contiguous(out_ap.ap[1:])
    assert ap_utils.ap_is_contiguous(idxs_ap.ap[1:])
    assert in_ap.ap[-1][1] == out_ap.ap[-1][1] == elem_size
    assert out_ap.ap[0][1] * out_ap.ap[1][1] == num_idxs and num_idxs % 128 == 0
    assert in_ap.ap[0][0] == elem_step
    stride_bytes = elem_step * mybir.dt.size(in_ap.dtype)
    stride_bytes_256 = stride_bytes // 256
    assert stride_bytes_256 * 256 == stride_bytes and stride_bytes_256 < 256
    _in_ap = gp.lower_ap_dma(in_ap, for_custom_bir_dma=True)
    _idxs_ap = gp.lower_ap(idxs_ap)
    _out_ap = gp.lower_ap(out_ap)
    return gp.add_instruction(
        mybir.InstDMAGatherAnt(
            name=gp.bass.get_next_instruction_name(),
            ins=[*_in_ap, _idxs_ap, gp.lower_val_access(gp.to_reg(num_idxs))],
            outs=[_out_ap],
            transpose=False, num_idxs=num_idxs, elem_size=elem_size,
            stride_bytes_256=stride_bytes_256, gen_mode=0, single_packet=True,
            queue_num=queue_num, sbuf_tokens_per_rank=0,
            sbuf_free_dim_per_rank=0, sbuf_free_dim_pad_per_rank=0,
            sbuf_byte_offset=0))


def _build(meta_c, meta_v, b_score_val):
    npk2_c, nraw_c = meta_c
    npk2_v, nraw_v = meta_v
    Tt_c = int((2 * npk2_c + nraw_c).sum())
    Tt_v = int((2 * npk2_v + nraw_v).sum())
    Mt_c = int((npk2_c + nraw_c).sum())
    Mt_v = int((npk2_v + nraw_v).sum())
    calls_c, units_c, bg_c = _plan(npk2_c, nraw_c, NB_C, VWIN)
    calls_v, units_v, bg_v = _plan(npk2_v, nraw_v, NB_V, CWIN)

    nc = bacc.Bacc("TRN2", target_bir_lowering=False, num_devices=CORES,
                   num_swdge_queues=4)
    AluOp = mybir.AluOpType
    Act = mybir.ActivationFunctionType

    def ein(name, shape, dtype):
        return nc.dram_tensor(name, shape, dtype, kind="ExternalInput")

    vfT = ein("vfT", [VF, V_S], F32)
    cfT = ein("cfT", [CF, C_S], F32)
    wvar = ein("wvar", [VF, H], F32)
    wcon = ein("wcon", [CF, H], F32)
    wv2c = ein("wv2c", [H, H], F32)
    wc2v = ein("wc2v", [H, H], F32)
    wsco = ein("wsco", [H, 1], BF16)
    bvar = ein("bvar", [H, 1], F32)
    bcon = ein("bcon", [H, 1], F32)
    bv2c = ein("bv2c", [H, 1], F32)
    bc2v = ein("bc2v", [H, 1], F32)
    idx_v2c_d = ein("idx_v2c", [P, Tt_c * 8], I16)
    idx_c2v_d = ein("idx_c2v", [P, Tt_v * 8], I16)
    pdst_v2c_d = ein("pdst_v2c", [P, Mt_c], BF16)
    pdst_c2v_d = ein("pdst_c2v", [P, Mt_v], BF16)
    iota_d = ein("iota", [P, P], BF16)
    ident_d = ein("ident", [H, H], BF16)
    scores_out = nc.dram_tensor("scores", [V_S], F32, kind="ExternalOutput")

    with tile.TileContext(nc) as tc:
        with (
            tc.tile_pool(name="const", bufs=1) as cpool,
            tc.tile_pool(name="state", bufs=1) as spool,
            tc.tile_pool(name="dram", bufs=1, space="DRAM") as dpool,
            tc.tile_pool(name="gpool", bufs=12) as gpool,
            tc.tile_pool(name="s_pool", bufs=6) as s_pool,
            tc.tile_pool(name="misc", bufs=4) as mpool,
            tc.tile_pool(name="ps_acc", bufs=4, space="PSUM") as ps_acc,
            tc.tile_pool(name="ps_upd", bufs=2, space="PSUM") as ps_upd,
            tc.tile_pool(name="ps_misc", bufs=2, space="PSUM") as ps_misc,
        ):
            def load_const(name, dram, shape, dtype):
                t = cpool.tile(shape, dtype, name=name)
                nc.sync.dma_start(out=t[:], in_=dram[:])
                return t

            iota_sb = load_const("iota_sb", iota_d, [P, P], BF16)
            ident_sb = load_const("ident_sb", ident_d, [H, H], BF16)
            wvar_sb = load_const("wvar_sb", wvar, [VF, H], F32)
            wcon_sb = load_const("wcon_sb", wcon, [CF, H], F32)
            wv2c_sb = load_const("wv2c_sb", wv2c, [H, H], F32)
            wc2v_sb = load_const("wc2v_sb", wc2v, [H, H], F32)
            wsco_sb = load_const("wsco_sb", wsco, [H, 1], BF16)
            bvar_sb = load_const("bvar_sb", bvar, [H, 1], F32)
            bcon_sb = load_const("bcon_sb", bcon, [H, 1], F32)
            bv2c_sb = load_const("bv2c_sb", bv2c, [H, 1], F32)
            bc2v_sb = load_const("bc2v_sb", bc2v, [H, 1], F32)
            idx_c_sb = load_const("idx_c_sb", idx_v2c_d, [P, Tt_c * 8], I16)
            idx_v_sb = load_const("idx_v_sb", idx_c2v_d, [P, Tt_v * 8], I16)
            pdst_c_sb = load_const("pdst_c_sb", pdst_v2c_d, [P, Mt_c], BF16)
            pdst_v_sb = load_const("pdst_v_sb", pdst_c2v_d, [P, Mt_v], BF16)

            hvT = spool.tile([H, V_S], BF16, name="hvT")
            hcT = spool.tile([H, C_S], BF16, name="hcT")
            macc = spool.tile([H, NB_V * P], F32, name="macc")
            rstage = spool.tile([P, NB_V * H], BF16, name="rstage")
            zrow_sb = cpool.tile([P, H], BF16, name="zrow_sb")
            nc.vector.memset(zrow_sb[:], 0.0)

            tabs_v = [dpool.tile([RV, H], BF16, name=f"tab_v{i}",
                                 addr_space="Shared", tag=f"tab_v{i}")
                      for i in range(ROUNDS)]
            tabs_c = [dpool.tile([RC, H], BF16, name=f"tab_c{i}",
                                 addr_space="Shared", tag=f"tab_c{i}")
                      for i in range(ROUNDS)]
            agin_v = dpool.tile([V_S, H], BF16, name="agin_v")
            agin_c = dpool.tile([C_S, H], BF16, name="agin_c")
            nc.sync.dma_start(out=agin_v[NB_V * P:V_S, :], in_=zrow_sb[:])
            nc.sync.dma_start(out=agin_c[NB_C * P:C_S, :], in_=zrow_sb[:])

            # ---- initial embeddings hT = relu(W.T @ featT + b)
            def emit_init(featT_dram, fdim, n_s, w_sb, b_sb, hT):
                with tc.tile_pool(name="initp", bufs=2) as ipool:
                    c0 = 0
                    while c0 < n_s:
                        wd = min(512, n_s - c0)
                        fch = ipool.tile([fdim, 512], F32, name="fch", tag="fch")
                        nc.sync.dma_start(out=fch[:, :wd],
                                          in_=featT_dram[:, c0:c0 + wd])
                        psi = ps_misc.tile([H, 512], F32, name="psi", tag="misc")
                        nc.tensor.matmul(out=psi[:, :wd], lhsT=w_sb[:],
                                         rhs=fch[:, :wd], start=True, stop=True)
                        nc.scalar.activation(out=hT[:, c0:c0 + wd], in_=psi[:, :wd],
                                             func=Act.Relu, bias=b_sb[:])
                        c0 += wd

            emit_init(vfT, VF, V_S, wvar_sb, bvar_sb, hvT)
            emit_init(cfT, CF, C_S, wcon_sb, bcon_sb, hcT)

            def emit_writeback(hT, nblk, agin, tab):
                for b in range(nblk):
                    psr = ps_misc.tile([P, H], BF16, name="psr", tag="misc")
                    nc.tensor.transpose(out=psr[:], in_=hT[:, b * P:(b + 1) * P],
                                        identity=ident_sb[:])
                    nc.vector.tensor_copy(out=rstage[:, b * H:(b + 1) * H],
                                          in_=psr[:])
                nc.sync.dma_start(
                    out=agin[0:nblk * P, :].rearrange("(b p) f -> p b f", p=P),
                    in_=rstage[:, :nblk * H].rearrange("p (b f) -> p b f", f=H))
                nc.gpsimd.collective_compute(
                    "AllGather", mybir.AluOpType.bypass,
                    replica_groups=[list(range(CORES))],
                    ins=[agin[:]], outs=[tab[:]])

            emit_writeback(hvT, NB_V, agin_v, tabs_v[0])

            # ---- one message-passing phase
            def emit_phase(tab_src, nrows_tab, nwin, idx_sb, pdst_sb,
                           calls, units, blk_groups, nblk, hT, W_sb, b_sb,
                           writeback):
                # even/odd row views of the compact table, per window
                tab2 = tab_src[:].rearrange("(n two) f -> n (two f)", two=2)
                whalf = (nrows_tab // nwin) // 2
                win_ap = {}
                for w in range(nwin):
                    for h in range(2):
                        win_ap[(w, h)] = tab2[w * whalf:(w + 1) * whalf,
                                              h * H:(h + 1) * H]

                accs = {}
                done_groups = [0] * nblk

                def emit_update(b):
                    ps2 = ps_upd.tile([H, P], F32, name="ps2", tag="ps2")
                    nc.tensor.matmul(out=ps2[:], lhsT=W_sb[:],
                                     rhs=macc[:, b * P:(b + 1) * P],
                                     start=True, stop=True)
                    tmp = mpool.tile([H, P], F32, name="tmp", tag="tmp")
                    nc.vector.tensor_tensor(out=tmp[:], in0=ps2[:],
                                            in1=hT[:, b * P:(b + 1) * P],
                                            op=AluOp.add)
                    nc.scalar.activation(out=hT[:, b * P:(b + 1) * P],
                                         in_=tmp[:], func=Act.Relu, bias=b_sb[:])

                for ci, (w, h, t0, mu0, ulist) in enumerate(calls):
                    nt = sum(2 if units[u][1] else 1 for u in ulist)
                    nmu = len(ulist)
                    g = gpool.tile([P, TPC, H], BF16, name="g", tag="g")
                    _dma_gather_raw(
                        nc.gpsimd, g[:, :nt, :], win_ap[(w, h)],
                        idx_sb[:, t0 * 8:(t0 + nt) * 8],
                        num_idxs=nt * P, elem_size=H, elem_step=2 * H,
                        queue_num=ci % 4)
                    S = s_pool.tile([P, TPC, P], BF16, name="S", tag="S")
                    nc.vector.tensor_tensor(
                        out=S[:, :nmu, :],
                        in0=iota_sb[:, None, :].to_broadcast([P, nmu, P]),
                        in1=pdst_sb[:, mu0:mu0 + nmu, None]
                            .to_broadcast([P, nmu, P]),
                        op=AluOp.is_equal)
                    off = 0
                    for mi, u in enumerate(ulist):
                        grp, is_pack, _, first, last = units[u]
                        if is_pack:
                            ss = mpool.tile([P, H], BF16, name="ss", tag="ss",
                                            bufs=4)
                            nc.vector.tensor_tensor(
                                out=ss[:], in0=g[:, off, :], in1=g[:, off + 1, :],
                                op=AluOp.add)
                            lhs = ss[:]
                            off += 2
                        else:
                            lhs = g[:, off, :]
                            off += 1
                        if first:
                            accs[grp] = ps_acc.tile([H, P], F32, name="acc",
                                                    tag="acc")
                        nc.tensor.matmul(out=accs[grp][:], lhsT=lhs,
                                         rhs=S[:, mi, :], start=first, stop=last)
                        if last:
                            b = grp % nblk
                            glist = blk_groups[b]
                            if done_groups[b] == 0:
                                nc.vector.tensor_copy(
                                    out=macc[:, b * P:(b + 1) * P],
                                    in_=accs[grp][:])
                            else:
                                nc.vector.tensor_tensor(
                                    out=macc[:, b * P:(b + 1) * P],
                                    in0=macc[:, b * P:(b + 1) * P],
                                    in1=accs[grp][:], op=AluOp.add)
                            del accs[grp]
                            done_groups[b] += 1
                            if done_groups[b] == len(glist):
                                emit_update(b)
                if writeback is not None:
                    writeback()

            for r in range(ROUNDS):
                emit_phase(tabs_v[r], RV, VWIN, idx_c_sb, pdst_c_sb,
                           calls_c, units_c, bg_c, NB_C, hcT, wv2c_sb, bv2c_sb,
                           lambda r=r: emit_writeback(hcT, NB_C, agin_c,
                                                      tabs_c[r]))
                last = r == ROUNDS - 1
                emit_phase(tabs_c[r], RC, CWIN, idx_v_sb, pdst_v_sb,
                           calls_v, units_v, bg_v, NB_V, hvT, wc2v_sb, bc2v_sb,
                           None if last else
                           (lambda r=r: emit_writeback(hvT, NB_V, agin_v,
                                                       tabs_v[r + 1])))

            # ---- scores = h_var @ w_score + b_score (shard)
            c0 = 0
            while c0 < V_S:
                wd = min(512, V_S - c0)
                pss = ps_misc.tile([1, 512], F32, name="pss", tag="misc")
                nc.tensor.matmul(out=pss[:, :wd], lhsT=wsco_sb[:],
                                 rhs=hvT[:, c0:c0 + wd], start=True, stop=True)
                sch = mpool.tile([1, 512], F32, name="sch", tag="sch")
                nc.vector.tensor_scalar(
                    out=sch[:, :wd], in0=pss[:, :wd],
                    scalar1=float(b_score_val), scalar2=None, op0=AluOp.add)
                nc.sync.dma_start(out=scores_out[None, c0:c0 + wd],
                                  in_=sch[0:1, :wd])
                c0 += wd

    nc.compile()
    return nc


_CACHE = {}


def kernel(**inputs):
    var_feat = np.asarray(inputs["var_feat"], np.float32)
    constr_feat = np.asarray(inputs["constr_feat"], np.float32)
    var_idx = np.asarray(inputs["var_idx"]).astype(np.int64)
    constr_idx = np.asarray(inputs["constr_idx"]).astype(np.int64)
    b_score_val = float(np.asarray(inputs["b_score"]).reshape(-1)[0])

    key = (var_idx.tobytes(), constr_idx.tobytes())
    if key in _CACHE:
        nc, idx_v, pdst_v, idx_c, pdst_c = _CACHE[key]
    else:
        # v2c: dst=constr, src=var
        idx_v, pdst_v, npk2_c, nraw_c = _prep_direction(
            constr_idx, var_idx, C_CORE, NB_C, VWIN, V_CORE, V_S)
        # c2v: dst=var, src=constr
        idx_c, pdst_c, npk2_v, nraw_v = _prep_direction(
            var_idx, constr_idx, V_CORE, NB_V, CWIN, C_CORE, C_S)
        nc = _build((npk2_c, nraw_c), (npk2_v, nraw_v), b_score_val)
        _CACHE[key] = (nc, idx_v, pdst_v, idx_c, pdst_c)

    iota = np.broadcast_to(np.arange(P, dtype=np.float32),
                           (P, P)).astype(BF).copy()
    ident = np.eye(H, dtype=np.float32).astype(BF)

    vf_pad = np.zeros((CORES, V_S, VF), np.float32)
    vf_pad[:, :V_CORE] = var_feat.reshape(CORES, V_CORE, VF)
    cf_pad = np.zeros((CORES, C_S, CF), np.float32)
    cf_pad[:, :C_CORE] = constr_feat.reshape(CORES, C_CORE, CF)

    common = dict(
        wvar=np.ascontiguousarray(inputs["W_var"], dtype=np.float32),
        wcon=np.ascontiguousarray(inputs["W_con"], dtype=np.float32),
        wv2c=np.ascontiguousarray(inputs["W_v2c"], dtype=np.float32),
        wc2v=np.ascontiguousarray(inputs["W_c2v"], dtype=np.float32),
        wsco=np.ascontiguousarray(inputs["W_score"], dtype=np.float32).astype(BF),
        bvar=np.ascontiguousarray(inputs["b_var"], dtype=np.float32).reshape(H, 1),
        bcon=np.ascontiguousarray(inputs["b_con"], dtype=np.float32).reshape(H, 1),
        bv2c=np.ascontiguousarray(inputs["b_v2c"], dtype=np.float32).reshape(H, 1),
        bc2v=np.ascontiguousarray(inputs["b_c2v"], dtype=np.float32).reshape(H, 1),
        iota=iota, ident=ident,
    )
    in_maps = []
    for k in range(CORES):
        m = dict(common)
        m["vfT"] = np.ascontiguousarray(vf_pad[k].T)
        m["cfT"] = np.ascontiguousarray(cf_pad[k].T)
        m["idx_v2c"] = idx_v[k]
        m["pdst_v2c"] = pdst_v[k]
        m["idx_c2v"] = idx_c[k]
        m["pdst_c2v"] = pdst_c[k]
        in_maps.append(m)

    res = run_bass_kernel_spmd(nc, in_maps, list(range(CORES)))
    scores = np.concatenate([res.results[k]["scores"].reshape(-1)[:V_CORE]
                             for k in range(CORES)])
    return scores.astype(np.float32)


# revision 26
# speedup vs baseline: 3.9250x; 1.1602x over previous
"""Trainium2 Bass kernel: bipartite GNN message passing (BranchingGNN), 8-core SPMD.

Sharding: core k owns constraint rows [k*6250,(k+1)*6250) and variable rows
[k*12500,(k+1)*12500); each core processes all edges targeting its shard, so
messages need no cross-core reduction. Node tables are compact row-major
[N, 64] bf16 in DRAM and are re-broadcast each phase by an AllGather of the
updated shard.

Per phase (one message direction):
  - edges sorted by (src-window, src-parity, dst-block, src-row); each
    (window,parity,block) group is padded to 128-edge tiles. Source rows are
    gathered by dma_gather (128B rows at 256B stride over the even/odd row
    subsequence of the compact table), ascending addresses for HBM locality.
  - per tile, a one-hot S [128e,128d] = (iota == dstcol) is built on DVE
    (batched per gather call); one PE matmul per tile accumulates
    msgT [64,128] into a per-group PSUM tile (start/stop over the group).
  - group partials are added into an SBUF accumulator macc per dst block;
    the block update relu(hT + W.T @ msgT + b) runs as soon as its last
    group lands; PE transpose + one DMA + AllGather republish the table.
"""
import sys

sys.path.insert(0, "/opt/trn_rl_repo")

import numpy as np
import ml_dtypes

import concourse.bass as bass
import concourse.bacc as bacc
import concourse.mybir as mybir
import concourse.tile as tile
from concourse.bass_utils import run_bass_kernel_spmd

# ---- problem constants
V, C, E = 100000, 50000, 1250000
VF, CF, H = 32, 32, 64
ROUNDS = 3
CORES = 8
P = 128
TPC = 7               # tiles per gather call (ring cap 64 descs)

V_CORE, C_CORE = 12500, 6250          # real nodes per core
V_S, C_S = 12544, 6272                # shard rows (98 / 49 blocks)
NB_V, NB_C = 98, 49                   # dst blocks per core
RV, RC = CORES * V_S, CORES * C_S     # 100352 / 50176 table rows
VWIN, CWIN = 2, 1                     # source windows (half-row reach 32767)
# chunk-major table layout: block ranges per table tensor (= gather window,
# = one AllGather each); plus finer DMA staging chunks
VCH = [0, 49, 98]
CCH = [0, 49]
VDM = [0, 25, 49, 74, 98]
CDM = [0, 13, 25, 37, 49]


def _row_map(n_core, chb):
    """node id -> chunk-major table row."""
    n = n_core * CORES
    v = np.arange(n, dtype=np.int64)
    k = v // n_core
    l = v % n_core
    b = l // P
    chb = np.asarray(chb, np.int64)
    c = np.searchsorted(chb, b, "right") - 1
    CR = (chb[1:] - chb[:-1]) * P
    crb = np.concatenate([[0], np.cumsum(CR)])[:-1]
    return 8 * crb[c] + k * CR[c] + (l - P * chb[c])

BF16 = mybir.dt.bfloat16
F32 = mybir.dt.float32
I16 = mybir.dt.int16
BF = ml_dtypes.bfloat16


def _prep_direction(dst, row, n_dst_core, nblk, nwin, wsize):
    """Edge metadata for one direction, pair-slot layout.

    Per (core, w, h, b) group: edges of each dst j are paired; full pairs
    (up to a min-over-cores pack budget) form 2-tile packs whose slots are
    pair-sums; leftovers go to raw 1-tile units (slot per edge). Returns
    (idx16 [CORES,128,Ttot*8], pdst [CORES,128,MUtot] bf16,
     npk2 [NG], nraw [NG]).
    """
    dst = np.asarray(dst, np.int64)
    row = np.asarray(row, np.int64)
    NE = dst.size

    w = row // wsize
    h = row % 2
    half = (row % wsize) // 2

    core = dst // n_dst_core
    dloc = dst % n_dst_core
    b = dloc // P
    j = dloc % P

    NG = nwin * 2 * nblk
    grp = (w * 2 + h) * nblk + b

    order = np.lexsort((half, j, grp, core))
    grp_s, core_s, half_s, j_s = grp[order], core[order], half[order], j[order]

    # rank within (core, grp, j) run
    rid = (core_s * NG + grp_s) * P + j_s
    rcnt = np.bincount(rid, minlength=CORES * NG * P)
    rstart = np.zeros(rcnt.size + 1, np.int64)
    rstart[1:] = np.cumsum(rcnt)
    rank = np.arange(NE, dtype=np.int64) - rstart[rid]
    nrun = rcnt[rid]
    is_pair = rank < (nrun - nrun % 2)

    # pair-slot number within (core, grp): pairs of earlier j-runs + own
    pj = rcnt // 2                                    # pairs per run
    pj_cg = pj.reshape(CORES * NG, P)
    pj_cum = np.cumsum(pj_cg, 1) - pj_cg              # pairs before run, in-grp
    pair_slot = pj_cum.reshape(-1)[rid] + rank // 2   # valid where is_pair
    elem = rank % 2

    S2 = pj_cg.sum(1).reshape(CORES, NG)              # pair slots per core/grp
    npk2 = (S2 // P).min(0)                           # full packs per grp
    in_pack = is_pair & (pair_slot < npk2[grp_s] * P)

    # raw-slot rank per (core, grp) among non-packed edges, keep sort order
    kcg = core_s * NG + grp_s
    raw_mask = ~in_pack
    raw_rank = np.zeros(NE, np.int64)
    kraw = kcg[raw_mask]
    o2 = np.argsort(kraw, kind="stable")
    cnt_raw = np.bincount(kraw, minlength=CORES * NG)
    st = np.zeros(CORES * NG + 1, np.int64)
    st[1:] = np.cumsum(cnt_raw)
    rr = np.empty(kraw.size, np.int64)
    rr[o2] = np.arange(kraw.size) - st[kraw[o2]]
    raw_rank[raw_mask] = rr
    nraw = -(-cnt_raw.reshape(CORES, NG).max(0) // P)

    T_g = 2 * npk2 + nraw                             # tiles per grp
    MU_g = npk2 + nraw                                # matmul units per grp
    TB = np.cumsum(T_g) - T_g
    MB = np.cumsum(MU_g) - MU_g
    Ttot, MUtot = int(T_g.sum()), int(MU_g.sum())

    # flat idx position and (mu, slot) per edge
    pk = pair_slot // P
    ps = pair_slot % P
    tpos = np.where(in_pack,
                    (TB[grp_s] + 2 * pk + elem) * P + ps,
                    (TB[grp_s] + 2 * npk2[grp_s] + raw_rank // P) * P
                    + raw_rank % P)
    mu = np.where(in_pack, MB[grp_s] + pk,
                  MB[grp_s] + npk2[grp_s] + raw_rank // P)
    mslot = np.where(in_pack, ps, raw_rank % P)

    idx16 = np.zeros((CORES, Ttot * P), np.int16)
    valid = np.zeros((CORES, Ttot * P), bool)
    idx16[core_s, tpos] = half_s.astype(np.int16)
    valid[core_s, tpos] = True
    pdst = np.full((CORES, MUtot * P), -1.0, np.float32)
    pdst[core_s, mu * P + mslot] = j_s

    for k in range(CORES):
        v = valid[k]
        pos = np.where(v, np.arange(Ttot * P), 0)
        np.maximum.accumulate(pos, out=pos)
        idx16[k] = idx16[k][pos]

    packed = np.zeros((CORES, P, Ttot * 8), np.int16)
    for k in range(CORES):
        a = idx16[k].reshape(-1, 16).T
        packed[k] = np.tile(a, (8, 1))

    pdst_t = pdst.reshape(CORES, MUtot, P).transpose(0, 2, 1)
    return packed, np.ascontiguousarray(pdst_t).astype(BF), npk2, nraw


def _plan(npk2, nraw, nblk, nwin):
    """Compile-time schedule.

    Returns (calls, units, blk_groups): calls = (w, h, t0, mu0, unit_idx
    list); units[u] = (grp, is_pack, tile_off_in_grp, first_mu, last_mu).
    """
    NG = len(npk2)
    T_g = 2 * npk2 + nraw
    MU_g = npk2 + nraw
    TB = np.cumsum(T_g) - T_g
    MB = np.cumsum(MU_g) - MU_g
    units = []
    for g in range(NG):
        for k in range(int(npk2[g])):
            units.append((g, True, 2 * k, k == 0,
                          k == int(MU_g[g]) - 1))
        for r in range(int(nraw[g])):
            units.append((g, False, 2 * int(npk2[g]) + r,
                          int(npk2[g]) + r == 0,
                          int(npk2[g]) + r == int(MU_g[g]) - 1))
    calls = []
    for wh in range(nwin * 2):
        g0, g1 = wh * nblk, (wh + 1) * nblk
        u = int(MB[g0])
        u_end = int(MB[g1 - 1] + MU_g[g1 - 1])
        while u < u_end:
            nt = 0
            ulist = []
            while u < u_end and nt + (2 if units[u][1] else 1) <= TPC:
                ulist.append(u)
                nt += 2 if units[u][1] else 1
                u += 1
            g0t, _, off0, _, _ = units[ulist[0]]
            t0 = int(TB[g0t]) + off0
            calls.append((wh // 2, wh % 2, t0, ulist[0], ulist))
    blk_groups = [[] for _ in range(nblk)]
    for g in range(NG):
        if MU_g[g] > 0:
            blk_groups[g % nblk].append(g)
    return calls, units, blk_groups


def _dma_gather_raw(gp, out_ap, in_ap, idxs_ap, num_idxs, elem_size, elem_step,
                    queue_num=0):
    """dma_gather (non-transpose, HBM source) allowing 128B rows at 256B stride."""
    from concourse import ap_utils
    gp._assert_queue_num(queue_num)
    assert idxs_ap.dtype == mybir.dt.int16
    assert in_ap.dtype == out_ap.dtype
    assert ap_utils.ap_is_contiguous(in_ap.ap[1:])
    assert ap_utils.ap_is_contiguous(out_ap.ap[1:])
    assert ap_utils.ap_is_contiguous(idxs_ap.ap[1:])
    assert in_ap.ap[-1][1] == out_ap.ap[-1][1] == elem_size
    assert out_ap.ap[0][1] * out_ap.ap[1][1] == num_idxs and num_idxs % 128 == 0
    assert in_ap.ap[0][0] == elem_step
    stride_bytes = elem_step * mybir.dt.size(in_ap.dtype)
    stride_bytes_256 = stride_bytes // 256
    assert stride_bytes_256 * 256 == stride_bytes and stride_bytes_256 < 256
    _in_ap = gp.lower_ap_dma(in_ap, for_custom_bir_dma=True)
    _idxs_ap = gp.lower_ap(idxs_ap)
    _out_ap = gp.lower_ap(out_ap)
    return gp.add_instruction(
        mybir.InstDMAGatherAnt(
            name=gp.bass.get_next_instruction_name(),
            ins=[*_in_ap, _idxs_ap, gp.lower_val_access(gp.to_reg(num_idxs))],
            outs=[_out_ap],
            transpose=False, num_idxs=num_idxs, elem_size=elem_size,
            stride_bytes_256=stride_bytes_256, gen_mode=0, single_packet=True,
            queue_num=queue_num, sbuf_tokens_per_rank=0,
            sbuf_free_dim_per_rank=0, sbuf_free_dim_pad_per_rank=0,
            sbuf_byte_offset=0))


def _build(meta_c, meta_v, b_score_val):
    npk2_c, nraw_c = meta_c
    npk2_v, nraw_v = meta_v
    Tt_c = int((2 * npk2_c + nraw_c).sum())
    Tt_v = int((2 * npk2_v + nraw_v).sum())
    Mt_c = int((npk2_c + nraw_c).sum())
    Mt_v = int((npk2_v + nraw_v).sum())
    calls_c, units_c, bg_c = _plan(npk2_c, nraw_c, NB_C, VWIN)
    calls_v, units_v, bg_v = _plan(npk2_v, nraw_v, NB_V, CWIN)

    nc = bacc.Bacc("TRN2", target_bir_lowering=False, num_devices=CORES,
                   num_swdge_queues=4)
    AluOp = mybir.AluOpType
    Act = mybir.ActivationFunctionType

    def ein(name, shape, dtype):
        return nc.dram_tensor(name, shape, dtype, kind="ExternalInput")

    vfT = ein("vfT", [VF, V_S], F32)
    cfT = ein("cfT", [CF, C_S], F32)
    wvar = ein("wvar", [VF, H], F32)
    wcon = ein("wcon", [CF, H], F32)
    wv2c = ein("wv2c", [H, H], F32)
    wc2v = ein("wc2v", [H, H], F32)
    wsco = ein("wsco", [H, 1], BF16)
    bvar = ein("bvar", [H, 1], F32)
    bcon = ein("bcon", [H, 1], F32)
    bv2c = ein("bv2c", [H, 1], F32)
    bc2v = ein("bc2v", [H, 1], F32)
    idx_v2c_d = ein("idx_v2c", [P, Tt_c * 8], I16)
    idx_c2v_d = ein("idx_c2v", [P, Tt_v * 8], I16)
    pdst_v2c_d = ein("pdst_v2c", [P, Mt_c], BF16)
    pdst_c2v_d = ein("pdst_c2v", [P, Mt_v], BF16)
    iota_d = ein("iota", [P, P], BF16)
    ident_d = ein("ident", [H, H], BF16)
    scores_out = nc.dram_tensor("scores", [V_S], F32, kind="ExternalOutput")

    with tile.TileContext(nc) as tc:
        with (
            tc.tile_pool(name="const", bufs=1) as cpool,
            tc.tile_pool(name="state", bufs=1) as spool,
            tc.tile_pool(name="dram", bufs=1, space="DRAM") as dpool,
            tc.tile_pool(name="gpool", bufs=12) as gpool,
            tc.tile_pool(name="s_pool", bufs=6) as s_pool,
            tc.tile_pool(name="misc", bufs=4) as mpool,
            tc.tile_pool(name="ps_acc", bufs=4, space="PSUM") as ps_acc,
            tc.tile_pool(name="ps_upd", bufs=2, space="PSUM") as ps_upd,
            tc.tile_pool(name="ps_misc", bufs=2, space="PSUM") as ps_misc,
        ):
            def load_const(name, dram, shape, dtype):
                t = cpool.tile(shape, dtype, name=name)
                nc.sync.dma_start(out=t[:], in_=dram[:])
                return t

            iota_sb = load_const("iota_sb", iota_d, [P, P], BF16)
            ident_sb = load_const("ident_sb", ident_d, [H, H], BF16)
            wvar_sb = load_const("wvar_sb", wvar, [VF, H], F32)
            wcon_sb = load_const("wcon_sb", wcon, [CF, H], F32)
            wv2c_sb = load_const("wv2c_sb", wv2c, [H, H], F32)
            wc2v_sb = load_const("wc2v_sb", wc2v, [H, H], F32)
            wsco_sb = load_const("wsco_sb", wsco, [H, 1], BF16)
            bvar_sb = load_const("bvar_sb", bvar, [H, 1], F32)
            bcon_sb = load_const("bcon_sb", bcon, [H, 1], F32)
            bv2c_sb = load_const("bv2c_sb", bv2c, [H, 1], F32)
            bc2v_sb = load_const("bc2v_sb", bc2v, [H, 1], F32)
            idx_c_sb = load_const("idx_c_sb", idx_v2c_d, [P, Tt_c * 8], I16)
            idx_v_sb = load_const("idx_v_sb", idx_c2v_d, [P, Tt_v * 8], I16)
            pdst_c_sb = load_const("pdst_c_sb", pdst_v2c_d, [P, Mt_c], BF16)
            pdst_v_sb = load_const("pdst_v_sb", pdst_c2v_d, [P, Mt_v], BF16)

            hvT = spool.tile([H, V_S], BF16, name="hvT")
            hcT = spool.tile([H, C_S], BF16, name="hcT")
            macc = spool.tile([H, NB_V * P], F32, name="macc")
            rstage = spool.tile([P, NB_V * H], BF16, name="rstage")

            tabs_v = [[dpool.tile([RV // VWIN, H], BF16, name=f"tab_v{i}_{w}",
                                  addr_space="Shared", tag=f"tab_v{i}_{w}")
                       for w in range(VWIN)] for i in range(ROUNDS)]
            tabs_c = [[dpool.tile([RC // CWIN, H], BF16, name=f"tab_c{i}_{w}",
                                  addr_space="Shared", tag=f"tab_c{i}_{w}")
                       for w in range(CWIN)] for i in range(ROUNDS)]
            agin_v = dpool.tile([V_S, H], BF16, name="agin_v")
            agin_c = dpool.tile([C_S, H], BF16, name="agin_c")

            # ---- initial embeddings hT = relu(W.T @ featT + b)
            def emit_init(featT_dram, fdim, n_s, w_sb, b_sb, hT):
                with tc.tile_pool(name="initp", bufs=2) as ipool:
                    c0 = 0
                    while c0 < n_s:
                        wd = min(512, n_s - c0)
                        fch = ipool.tile([fdim, 512], F32, name="fch", tag="fch")
                        nc.sync.dma_start(out=fch[:, :wd],
                                          in_=featT_dram[:, c0:c0 + wd])
                        psi = ps_misc.tile([H, 512], F32, name="psi", tag="misc")
                        nc.tensor.matmul(out=psi[:, :wd], lhsT=w_sb[:],
                                         rhs=fch[:, :wd], start=True, stop=True)
                        nc.scalar.activation(out=hT[:, c0:c0 + wd], in_=psi[:, :wd],
                                             func=Act.Relu, bias=b_sb[:])
                        c0 += wd

            emit_init(vfT, VF, V_S, wvar_sb, bvar_sb, hvT)
            emit_init(cfT, CF, C_S, wcon_sb, bcon_sb, hcT)

            def emit_chunk_dma(agin, b0, b1):
                nc.sync.dma_start(
                    out=agin[b0 * P:b1 * P, :].rearrange("(b p) f -> p b f", p=P),
                    in_=rstage[:, b0 * H:b1 * H].rearrange("p (b f) -> p b f",
                                                           f=H))

            def emit_chunk_colls(agin, tabs, coll_chb):
                for c in range(len(coll_chb) - 1):
                    b0, b1 = coll_chb[c], coll_chb[c + 1]
                    nc.gpsimd.collective_compute(
                        "AllGather", mybir.AluOpType.bypass,
                        replica_groups=[list(range(CORES))],
                        ins=[agin[b0 * P:b1 * P, :]],
                        outs=[tabs[c][:]])

            def emit_writeback(hT, nblk, agin, tabs, coll_chb, dma_chb):
                for b in range(nblk):
                    psr = ps_misc.tile([P, H], BF16, name="psr", tag="misc")
                    nc.tensor.transpose(out=psr[:], in_=hT[:, b * P:(b + 1) * P],
                                        identity=ident_sb[:])
                    nc.vector.tensor_copy(out=rstage[:, b * H:(b + 1) * H],
                                          in_=psr[:])
                for c in range(len(dma_chb) - 1):
                    emit_chunk_dma(agin, dma_chb[c], dma_chb[c + 1])
                emit_chunk_colls(agin, tabs, coll_chb)

            emit_writeback(hvT, NB_V, agin_v, tabs_v[0], VCH, VDM)

            # ---- one message-passing phase
            def emit_phase(tab_srcs, nwin, idx_sb, pdst_sb,
                           calls, units, blk_groups, nblk, hT, W_sb, b_sb,
                           wb):
                # even/odd row views of the per-window table tensors
                win_ap = {}
                for w in range(nwin):
                    tab2 = tab_srcs[w][:].rearrange("(n two) f -> n (two f)",
                                                    two=2)
                    for h in range(2):
                        win_ap[(w, h)] = tab2[:, h * H:(h + 1) * H]

                accs = {}
                done_groups = [0] * nblk
                if wb is not None:
                    agin, tabs, coll_chb, chb = wb
                    chunk_left = [chb[c + 1] - chb[c]
                                  for c in range(len(chb) - 1)]

                def emit_update(b):
                    ps2 = ps_upd.tile([H, P], F32, name="ps2", tag="ps2")
                    nc.tensor.matmul(out=ps2[:], lhsT=W_sb[:],
                                     rhs=macc[:, b * P:(b + 1) * P],
                                     start=True, stop=True)
                    tmp = mpool.tile([H, P], F32, name="tmp", tag="tmp")
                    nc.vector.tensor_tensor(out=tmp[:], in0=ps2[:],
                                            in1=hT[:, b * P:(b + 1) * P],
                                            op=AluOp.add)
                    nc.scalar.activation(out=hT[:, b * P:(b + 1) * P],
                                         in_=tmp[:], func=Act.Relu, bias=b_sb[:])
                    if wb is not None:
                        psr = ps_misc.tile([P, H], BF16, name="psr", tag="misc")
                        nc.tensor.transpose(out=psr[:],
                                            in_=hT[:, b * P:(b + 1) * P],
                                            identity=ident_sb[:])
                        nc.vector.tensor_copy(out=rstage[:, b * H:(b + 1) * H],
                                              in_=psr[:])
                        c = int(np.searchsorted(chb, b, "right")) - 1
                        chunk_left[c] -= 1
                        if chunk_left[c] == 0:
                            emit_chunk_dma(agin, chb[c], chb[c + 1])

                for ci, (w, h, t0, mu0, ulist) in enumerate(calls):
                    nt = sum(2 if units[u][1] else 1 for u in ulist)
                    nmu = len(ulist)
                    g = gpool.tile([P, TPC, H], BF16, name="g", tag="g")
                    _dma_gather_raw(
                        nc.gpsimd, g[:, :nt, :], win_ap[(w, h)],
                        idx_sb[:, t0 * 8:(t0 + nt) * 8],
                        num_idxs=nt * P, elem_size=H, elem_step=2 * H,
                        queue_num=ci % 4)
                    S = s_pool.tile([P, TPC, P], BF16, name="S", tag="S")
                    nc.vector.tensor_tensor(
                        out=S[:, :nmu, :],
                        in0=iota_sb[:, None, :].to_broadcast([P, nmu, P]),
                        in1=pdst_sb[:, mu0:mu0 + nmu, None]
                            .to_broadcast([P, nmu, P]),
                        op=AluOp.is_equal)
                    off = 0
                    for mi, u in enumerate(ulist):
                        grp, is_pack, _, first, last = units[u]
                        if is_pack:
                            ss = mpool.tile([P, H], BF16, name="ss", tag="ss",
                                            bufs=4)
                            nc.vector.tensor_tensor(
                                out=ss[:], in0=g[:, off, :], in1=g[:, off + 1, :],
                                op=AluOp.add)
                            lhs = ss[:]
                            off += 2
                        else:
                            lhs = g[:, off, :]
                            off += 1
                        if first:
                            accs[grp] = ps_acc.tile([H, P], F32, name="acc",
                                                    tag="acc")
                        nc.tensor.matmul(out=accs[grp][:], lhsT=lhs,
                                         rhs=S[:, mi, :], start=first, stop=last)
                        if last:
                            b = grp % nblk
                            glist = blk_groups[b]
                            if done_groups[b] == 0:
                                nc.vector.tensor_copy(
                                    out=macc[:, b * P:(b + 1) * P],
                                    in_=accs[grp][:])
                            else:
                                nc.vector.tensor_tensor(
                                    out=macc[:, b * P:(b + 1) * P],
                                    in0=macc[:, b * P:(b + 1) * P],
                                    in1=accs[grp][:], op=AluOp.add)
                            del accs[grp]
                            done_groups[b] += 1
                            if done_groups[b] == len(glist):
                                emit_update(b)
                if wb is not None:
                    emit_chunk_colls(agin, tabs, coll_chb)

            for r in range(ROUNDS):
                emit_phase(tabs_v[r], VWIN, idx_c_sb, pdst_c_sb,
                           calls_c, units_c, bg_c, NB_C, hcT, wv2c_sb, bv2c_sb,
                           (agin_c, tabs_c[r], CCH, CDM))
                last = r == ROUNDS - 1
                emit_phase(tabs_c[r], CWIN, idx_v_sb, pdst_v_sb,
                           calls_v, units_v, bg_v, NB_V, hvT, wc2v_sb, bc2v_sb,
                           None if last else
                           (agin_v, tabs_v[r + 1], VCH, VDM))

            # ---- scores = h_var @ w_score + b_score (shard)
            c0 = 0
            while c0 < V_S:
                wd = min(512, V_S - c0)
                pss = ps_misc.tile([1, 512], F32, name="pss", tag="misc")
                nc.tensor.matmul(out=pss[:, :wd], lhsT=wsco_sb[:],
                                 rhs=hvT[:, c0:c0 + wd], start=True, stop=True)
                sch = mpool.tile([1, 512], F32, name="sch", tag="sch")
                nc.vector.tensor_scalar(
                    out=sch[:, :wd], in0=pss[:, :wd],
                    scalar1=float(b_score_val), scalar2=None, op0=AluOp.add)
                nc.sync.dma_start(out=scores_out[None, c0:c0 + wd],
                                  in_=sch[0:1, :wd])
                c0 += wd

    nc.compile()
    return nc


_CACHE = {}


def kernel(**inputs):
    var_feat = np.asarray(inputs["var_feat"], np.float32)
    constr_feat = np.asarray(inputs["constr_feat"], np.float32)
    var_idx = np.asarray(inputs["var_idx"]).astype(np.int64)
    constr_idx = np.asarray(inputs["constr_idx"]).astype(np.int64)
    b_score_val = float(np.asarray(inputs["b_score"]).reshape(-1)[0])

    key = (var_idx.tobytes(), constr_idx.tobytes())
    if key in _CACHE:
        nc, idx_v, pdst_v, idx_c, pdst_c = _CACHE[key]
    else:
        rm_v = _row_map(V_CORE, VCH)
        rm_c = _row_map(C_CORE, CCH)
        # v2c: dst=constr, src=var
        idx_v, pdst_v, npk2_c, nraw_c = _prep_direction(
            constr_idx, rm_v[var_idx], C_CORE, NB_C, VWIN, RV // VWIN)
        # c2v: dst=var, src=constr
        idx_c, pdst_c, npk2_v, nraw_v = _prep_direction(
            var_idx, rm_c[constr_idx], V_CORE, NB_V, CWIN, RC // CWIN)
        nc = _build((npk2_c, nraw_c), (npk2_v, nraw_v), b_score_val)
        _CACHE[key] = (nc, idx_v, pdst_v, idx_c, pdst_c)

    iota = np.broadcast_to(np.arange(P, dtype=np.float32),
                           (P, P)).astype(BF).copy()
    ident = np.eye(H, dtype=np.float32).astype(BF)

    vf_pad = np.zeros((CORES, V_S, VF), np.float32)
    vf_pad[:, :V_CORE] = var_feat.reshape(CORES, V_CORE, VF)
    cf_pad = np.zeros((CORES, C_S, CF), np.float32)
    cf_pad[:, :C_CORE] = constr_feat.reshape(CORES, C_CORE, CF)

    common = dict(
        wvar=np.ascontiguousarray(inputs["W_var"], dtype=np.float32),
        wcon=np.ascontiguousarray(inputs["W_con"], dtype=np.float32),
        wv2c=np.ascontiguousarray(inputs["W_v2c"], dtype=np.float32),
        wc2v=np.ascontiguousarray(inputs["W_c2v"], dtype=np.float32),
        wsco=np.ascontiguousarray(inputs["W_score"], dtype=np.float32).astype(BF),
        bvar=np.ascontiguousarray(inputs["b_var"], dtype=np.float32).reshape(H, 1),
        bcon=np.ascontiguousarray(inputs["b_con"], dtype=np.float32).reshape(H, 1),
        bv2c=np.ascontiguousarray(inputs["b_v2c"], dtype=np.float32).reshape(H, 1),
        bc2v=np.ascontiguousarray(inputs["b_c2v"], dtype=np.float32).reshape(H, 1),
        iota=iota, ident=ident,
    )
    in_maps = []
    for k in range(CORES):
        m = dict(common)
        m["vfT"] = np.ascontiguousarray(vf_pad[k].T)
        m["cfT"] = np.ascontiguousarray(cf_pad[k].T)
        m["idx_v2c"] = idx_v[k]
        m["pdst_v2c"] = pdst_v[k]
        m["idx_c2v"] = idx_c[k]
        m["pdst_c2v"] = pdst_c[k]
        in_maps.append(m)

    res = run_bass_kernel_spmd(nc, in_maps, list(range(CORES)))
    scores = np.concatenate([res.results[k]["scores"].reshape(-1)[:V_CORE]
                             for k in range(CORES)])
    return scores.astype(np.float32)


# revision 29
# speedup vs baseline: 3.9559x; 1.0079x over previous
"""Trainium2 Bass kernel: bipartite GNN message passing (BranchingGNN), 8-core SPMD.

Sharding: core k owns constraint rows [k*6250,(k+1)*6250) and variable rows
[k*12500,(k+1)*12500); each core processes all edges targeting its shard, so
messages need no cross-core reduction. Node tables are compact row-major
[N, 64] bf16 in DRAM and are re-broadcast each phase by an AllGather of the
updated shard.

Per phase (one message direction):
  - edges sorted by (src-window, src-parity, dst-block, src-row); each
    (window,parity,block) group is padded to 128-edge tiles. Source rows are
    gathered by dma_gather (128B rows at 256B stride over the even/odd row
    subsequence of the compact table), ascending addresses for HBM locality.
  - per tile, a one-hot S [128e,128d] = (iota == dstcol) is built on DVE
    (batched per gather call); one PE matmul per tile accumulates
    msgT [64,128] into a per-group PSUM tile (start/stop over the group).
  - group partials are added into an SBUF accumulator macc per dst block;
    the block update relu(hT + W.T @ msgT + b) runs as soon as its last
    group lands; PE transpose + one DMA + AllGather republish the table.
"""
import sys

sys.path.insert(0, "/opt/trn_rl_repo")

import numpy as np
import ml_dtypes

import concourse.bass as bass
import concourse.bacc as bacc
import concourse.mybir as mybir
import concourse.tile as tile
from concourse.bass_utils import run_bass_kernel_spmd

# ---- problem constants
V, C, E = 100000, 50000, 1250000
VF, CF, H = 32, 32, 64
ROUNDS = 3
CORES = 8
P = 128
TPC = 7               # tiles per gather call (ring cap 64 descs)

V_CORE, C_CORE = 12500, 6250          # real nodes per core
V_S, C_S = 12544, 6272                # shard rows (98 / 49 blocks)
NB_V, NB_C = 98, 49                   # dst blocks per core
RV, RC = CORES * V_S, CORES * C_S     # 100352 / 50176 table rows
VWIN, CWIN = 2, 1                     # source windows (half-row reach 32767)
# chunk-major table layout: block ranges per table tensor (= gather window,
# = one AllGather each); plus finer DMA staging chunks
VCH = [0, 49, 98]
CCH = [0, 49]
VDM = [0, 25, 49, 74, 98]
CDM = [0, 13, 25, 37, 49]


def _row_map(n_core, chb):
    """node id -> chunk-major table row."""
    n = n_core * CORES
    v = np.arange(n, dtype=np.int64)
    k = v // n_core
    l = v % n_core
    b = l // P
    chb = np.asarray(chb, np.int64)
    c = np.searchsorted(chb, b, "right") - 1
    CR = (chb[1:] - chb[:-1]) * P
    crb = np.concatenate([[0], np.cumsum(CR)])[:-1]
    return 8 * crb[c] + k * CR[c] + (l - P * chb[c])

BF16 = mybir.dt.bfloat16
F32 = mybir.dt.float32
I16 = mybir.dt.int16
BF = ml_dtypes.bfloat16


def _prep_direction(dst, row, n_dst_core, nblk, nwin, wsize):
    """Edge metadata for one direction, pair-slot layout.

    Per (core, w, h, b) group: edges of each dst j are paired; full pairs
    (up to a min-over-cores pack budget) form 2-tile packs whose slots are
    pair-sums; leftovers go to raw 1-tile units (slot per edge). Returns
    (idx16 [CORES,128,Ttot*8], pdst [CORES,128,MUtot] bf16,
     npk2 [NG], nraw [NG]).
    """
    dst = np.asarray(dst, np.int64)
    row = np.asarray(row, np.int64)
    NE = dst.size

    w = row // wsize
    h = row % 2
    half = (row % wsize) // 2

    core = dst // n_dst_core
    dloc = dst % n_dst_core
    b = dloc // P
    j = dloc % P

    NG = nwin * 2 * nblk
    grp = (w * 2 + h) * nblk + b

    order = np.lexsort((half, j, grp, core))
    grp_s, core_s, half_s, j_s = grp[order], core[order], half[order], j[order]

    # rank within (core, grp, j) run
    rid = (core_s * NG + grp_s) * P + j_s
    rcnt = np.bincount(rid, minlength=CORES * NG * P)
    rstart = np.zeros(rcnt.size + 1, np.int64)
    rstart[1:] = np.cumsum(rcnt)
    rank = np.arange(NE, dtype=np.int64) - rstart[rid]
    nrun = rcnt[rid]
    is_pair = rank < (nrun - nrun % 2)

    # pair-slot number within (core, grp): pairs of earlier j-runs + own
    pj = rcnt // 2                                    # pairs per run
    pj_cg = pj.reshape(CORES * NG, P)
    pj_cum = np.cumsum(pj_cg, 1) - pj_cg              # pairs before run, in-grp
    pair_slot = pj_cum.reshape(-1)[rid] + rank // 2   # valid where is_pair
    elem = rank % 2

    S2 = pj_cg.sum(1).reshape(CORES, NG)              # pair slots per core/grp
    npk2 = (S2 // P).min(0)                           # full packs per grp
    in_pack = is_pair & (pair_slot < npk2[grp_s] * P)

    # raw-slot rank per (core, grp) among non-packed edges, keep sort order
    kcg = core_s * NG + grp_s
    raw_mask = ~in_pack
    raw_rank = np.zeros(NE, np.int64)
    kraw = kcg[raw_mask]
    o2 = np.argsort(kraw, kind="stable")
    cnt_raw = np.bincount(kraw, minlength=CORES * NG)
    st = np.zeros(CORES * NG + 1, np.int64)
    st[1:] = np.cumsum(cnt_raw)
    rr = np.empty(kraw.size, np.int64)
    rr[o2] = np.arange(kraw.size) - st[kraw[o2]]
    raw_rank[raw_mask] = rr
    nraw = -(-cnt_raw.reshape(CORES, NG).max(0) // P)

    T_g = 2 * npk2 + nraw                             # tiles per grp
    MU_g = npk2 + nraw                                # matmul units per grp
    TB = np.cumsum(T_g) - T_g
    MB = np.cumsum(MU_g) - MU_g
    Ttot, MUtot = int(T_g.sum()), int(MU_g.sum())

    # flat idx position and (mu, slot) per edge
    pk = pair_slot // P
    ps = pair_slot % P
    tpos = np.where(in_pack,
                    (TB[grp_s] + 2 * pk + elem) * P + ps,
                    (TB[grp_s] + 2 * npk2[grp_s] + raw_rank // P) * P
                    + raw_rank % P)
    mu = np.where(in_pack, MB[grp_s] + pk,
                  MB[grp_s] + npk2[grp_s] + raw_rank // P)
    mslot = np.where(in_pack, ps, raw_rank % P)

    idx16 = np.zeros((CORES, Ttot * P), np.int16)
    valid = np.zeros((CORES, Ttot * P), bool)
    idx16[core_s, tpos] = half_s.astype(np.int16)
    valid[core_s, tpos] = True
    pdst = np.full((CORES, MUtot * P), -1.0, np.float32)
    pdst[core_s, mu * P + mslot] = j_s

    for k in range(CORES):
        v = valid[k]
        pos = np.where(v, np.arange(Ttot * P), 0)
        np.maximum.accumulate(pos, out=pos)
        idx16[k] = idx16[k][pos]

    packed = np.zeros((CORES, P, Ttot * 8), np.int16)
    for k in range(CORES):
        a = idx16[k].reshape(-1, 16).T
        packed[k] = np.tile(a, (8, 1))

    pdst_t = pdst.reshape(CORES, MUtot, P).transpose(0, 2, 1)
    return packed, np.ascontiguousarray(pdst_t).astype(BF), npk2, nraw


def _plan(npk2, nraw, nblk, nwin):
    """Compile-time schedule.

    Returns (calls, units, blk_groups): calls = (w, h, t0, mu0, unit_idx
    list); units[u] = (grp, is_pack, tile_off_in_grp, first_mu, last_mu).
    """
    NG = len(npk2)
    T_g = 2 * npk2 + nraw
    MU_g = npk2 + nraw
    TB = np.cumsum(T_g) - T_g
    MB = np.cumsum(MU_g) - MU_g
    units = []
    for g in range(NG):
        for k in range(int(npk2[g])):
            units.append((g, True, 2 * k, k == 0,
                          k == int(MU_g[g]) - 1))
        for r in range(int(nraw[g])):
            units.append((g, False, 2 * int(npk2[g]) + r,
                          int(npk2[g]) + r == 0,
                          int(npk2[g]) + r == int(MU_g[g]) - 1))
    calls = []
    for wh in range(nwin * 2):
        g0, g1 = wh * nblk, (wh + 1) * nblk
        u = int(MB[g0])
        u_end = int(MB[g1 - 1] + MU_g[g1 - 1])
        while u < u_end:
            nt = 0
            ulist = []
            while u < u_end and nt + (2 if units[u][1] else 1) <= TPC:
                ulist.append(u)
                nt += 2 if units[u][1] else 1
                u += 1
            g0t, _, off0, _, _ = units[ulist[0]]
            t0 = int(TB[g0t]) + off0
            calls.append((wh // 2, wh % 2, t0, ulist[0], ulist))
    blk_groups = [[] for _ in range(nblk)]
    for g in range(NG):
        if MU_g[g] > 0:
            blk_groups[g % nblk].append(g)
    return calls, units, blk_groups


def _dma_gather_raw(gp, out_ap, in_ap, idxs_ap, num_idxs, elem_size, elem_step,
                    queue_num=0):
    """dma_gather (non-transpose, HBM source) allowing 128B rows at 256B stride."""
    from concourse import ap_utils
    gp._assert_queue_num(queue_num)
    assert idxs_ap.dtype == mybir.dt.int16
    assert in_ap.dtype == out_ap.dtype
    assert ap_utils.ap_is_contiguous(in_ap.ap[1:])
    assert ap_utils.ap_is_contiguous(out_ap.ap[1:])
    assert ap_utils.ap_is_contiguous(idxs_ap.ap[1:])
    assert in_ap.ap[-1][1] == out_ap.ap[-1][1] == elem_size
    assert out_ap.ap[0][1] * out_ap.ap[1][1] == num_idxs and num_idxs % 128 == 0
    assert in_ap.ap[0][0] == elem_step
    stride_bytes = elem_step * mybir.dt.size(in_ap.dtype)
    stride_bytes_256 = stride_bytes // 256
    assert stride_bytes_256 * 256 == stride_bytes and stride_bytes_256 < 256
    _in_ap = gp.lower_ap_dma(in_ap, for_custom_bir_dma=True)
    _idxs_ap = gp.lower_ap(idxs_ap)
    _out_ap = gp.lower_ap(out_ap)
    return gp.add_instruction(
        mybir.InstDMAGatherAnt(
            name=gp.bass.get_next_instruction_name(),
            ins=[*_in_ap, _idxs_ap, gp.lower_val_access(gp.to_reg(num_idxs))],
            outs=[_out_ap],
            transpose=False, num_idxs=num_idxs, elem_size=elem_size,
            stride_bytes_256=stride_bytes_256, gen_mode=0, single_packet=True,
            queue_num=queue_num, sbuf_tokens_per_rank=0,
            sbuf_free_dim_per_rank=0, sbuf_free_dim_pad_per_rank=0,
            sbuf_byte_offset=0))


def _build(meta_c, meta_v, b_score_val):
    npk2_c, nraw_c = meta_c
    npk2_v, nraw_v = meta_v
    Tt_c = int((2 * npk2_c + nraw_c).sum())
    Tt_v = int((2 * npk2_v + nraw_v).sum())
    Mt_c = int((npk2_c + nraw_c).sum())
    Mt_v = int((npk2_v + nraw_v).sum())
    calls_c, units_c, bg_c = _plan(npk2_c, nraw_c, NB_C, VWIN)
    calls_v, units_v, bg_v = _plan(npk2_v, nraw_v, NB_V, CWIN)

    nc = bacc.Bacc("TRN2", target_bir_lowering=False, num_devices=CORES,
                   num_swdge_queues=4)
    AluOp = mybir.AluOpType
    Act = mybir.ActivationFunctionType

    def ein(name, shape, dtype):
        return nc.dram_tensor(name, shape, dtype, kind="ExternalInput")

    vfT = ein("vfT", [VF, V_S], F32)
    cfT = ein("cfT", [CF, C_S], F32)
    wvar = ein("wvar", [VF, H], F32)
    wcon = ein("wcon", [CF, H], F32)
    wv2c = ein("wv2c", [H, H], F32)
    wc2v = ein("wc2v", [H, H], F32)
    wsco = ein("wsco", [H, 1], BF16)
    bvar = ein("bvar", [H, 1], F32)
    bcon = ein("bcon", [H, 1], F32)
    bv2c = ein("bv2c", [H, 1], F32)
    bc2v = ein("bc2v", [H, 1], F32)
    idx_v2c_d = ein("idx_v2c", [P, Tt_c * 8], I16)
    idx_c2v_d = ein("idx_c2v", [P, Tt_v * 8], I16)
    pdst_v2c_d = ein("pdst_v2c", [P, Mt_c], BF16)
    pdst_c2v_d = ein("pdst_c2v", [P, Mt_v], BF16)
    iota_d = ein("iota", [P, P], BF16)
    ident_d = ein("ident", [H, H], BF16)
    scores_out = nc.dram_tensor("scores", [V_S], F32, kind="ExternalOutput")

    with tile.TileContext(nc) as tc:
        with (
            tc.tile_pool(name="const", bufs=1) as cpool,
            tc.tile_pool(name="state", bufs=1) as spool,
            tc.tile_pool(name="dram", bufs=1, space="DRAM") as dpool,
            tc.tile_pool(name="gpool", bufs=24) as gpool,
            tc.tile_pool(name="s_pool", bufs=10) as s_pool,
            tc.tile_pool(name="misc", bufs=4) as mpool,
            tc.tile_pool(name="ps_acc", bufs=4, space="PSUM") as ps_acc,
            tc.tile_pool(name="ps_upd", bufs=2, space="PSUM") as ps_upd,
            tc.tile_pool(name="ps_misc", bufs=2, space="PSUM") as ps_misc,
        ):
            def load_const(name, dram, shape, dtype):
                t = cpool.tile(shape, dtype, name=name)
                nc.sync.dma_start(out=t[:], in_=dram[:])
                return t

            iota_sb = load_const("iota_sb", iota_d, [P, P], BF16)
            ident_sb = load_const("ident_sb", ident_d, [H, H], BF16)
            wvar_sb = load_const("wvar_sb", wvar, [VF, H], F32)
            wcon_sb = load_const("wcon_sb", wcon, [CF, H], F32)
            wv2c_sb = load_const("wv2c_sb", wv2c, [H, H], F32)
            wc2v_sb = load_const("wc2v_sb", wc2v, [H, H], F32)
            wsco_sb = load_const("wsco_sb", wsco, [H, 1], BF16)
            bvar_sb = load_const("bvar_sb", bvar, [H, 1], F32)
            bcon_sb = load_const("bcon_sb", bcon, [H, 1], F32)
            bv2c_sb = load_const("bv2c_sb", bv2c, [H, 1], F32)
            bc2v_sb = load_const("bc2v_sb", bc2v, [H, 1], F32)
            idx_c_sb = load_const("idx_c_sb", idx_v2c_d, [P, Tt_c * 8], I16)
            idx_v_sb = load_const("idx_v_sb", idx_c2v_d, [P, Tt_v * 8], I16)
            pdst_c_sb = load_const("pdst_c_sb", pdst_v2c_d, [P, Mt_c], BF16)
            pdst_v_sb = load_const("pdst_v_sb", pdst_c2v_d, [P, Mt_v], BF16)

            hvT = spool.tile([H, V_S], BF16, name="hvT")
            hcT = spool.tile([H, C_S], BF16, name="hcT")
            macc = spool.tile([H, NB_V * P], F32, name="macc")
            rstage = spool.tile([P, NB_V * H], BF16, name="rstage")

            tabs_v = [[dpool.tile([RV // VWIN, H], BF16, name=f"tab_v{i}_{w}",
                                  addr_space="Shared", tag=f"tab_v{i}_{w}")
                       for w in range(VWIN)] for i in range(ROUNDS)]
            tabs_c = [[dpool.tile([RC // CWIN, H], BF16, name=f"tab_c{i}_{w}",
                                  addr_space="Shared", tag=f"tab_c{i}_{w}")
                       for w in range(CWIN)] for i in range(ROUNDS)]
            agin_v = dpool.tile([V_S, H], BF16, name="agin_v")
            agin_c = dpool.tile([C_S, H], BF16, name="agin_c")

            # ---- initial embeddings hT = relu(W.T @ featT + b)
            def emit_init(featT_dram, fdim, n_s, w_sb, b_sb, hT):
                with tc.tile_pool(name="initp", bufs=2) as ipool:
                    c0 = 0
                    while c0 < n_s:
                        wd = min(512, n_s - c0)
                        fch = ipool.tile([fdim, 512], F32, name="fch", tag="fch")
                        nc.sync.dma_start(out=fch[:, :wd],
                                          in_=featT_dram[:, c0:c0 + wd])
                        psi = ps_misc.tile([H, 512], F32, name="psi", tag="misc")
                        nc.tensor.matmul(out=psi[:, :wd], lhsT=w_sb[:],
                                         rhs=fch[:, :wd], start=True, stop=True)
                        nc.scalar.activation(out=hT[:, c0:c0 + wd], in_=psi[:, :wd],
                                             func=Act.Relu, bias=b_sb[:])
                        c0 += wd

            emit_init(vfT, VF, V_S, wvar_sb, bvar_sb, hvT)

            def emit_chunk_dma(agin, b0, b1):
                nc.sync.dma_start(
                    out=agin[b0 * P:b1 * P, :].rearrange("(b p) f -> p b f", p=P),
                    in_=rstage[:, b0 * H:b1 * H].rearrange("p (b f) -> p b f",
                                                           f=H))

            def emit_chunk_colls(agin, tabs, coll_chb):
                for c in range(len(coll_chb) - 1):
                    b0, b1 = coll_chb[c], coll_chb[c + 1]
                    nc.gpsimd.collective_compute(
                        "AllGather", mybir.AluOpType.bypass,
                        replica_groups=[list(range(CORES))],
                        ins=[agin[b0 * P:b1 * P, :]],
                        outs=[tabs[c][:]])

            def emit_writeback(hT, nblk, agin, tabs, coll_chb, dma_chb):
                for b in range(nblk):
                    psr = ps_misc.tile([P, H], BF16, name="psr", tag="misc")
                    nc.tensor.transpose(out=psr[:], in_=hT[:, b * P:(b + 1) * P],
                                        identity=ident_sb[:])
                    nc.vector.tensor_copy(out=rstage[:, b * H:(b + 1) * H],
                                          in_=psr[:])
                for c in range(len(dma_chb) - 1):
                    emit_chunk_dma(agin, dma_chb[c], dma_chb[c + 1])
                emit_chunk_colls(agin, tabs, coll_chb)

            emit_writeback(hvT, NB_V, agin_v, tabs_v[0], VCH, VDM)
            emit_init(cfT, CF, C_S, wcon_sb, bcon_sb, hcT)

            # ---- one message-passing phase
            def emit_phase(tab_srcs, nwin, idx_sb, pdst_sb,
                           calls, units, blk_groups, nblk, hT, W_sb, b_sb,
                           wb):
                # even/odd row views of the per-window table tensors
                win_ap = {}
                for w in range(nwin):
                    tab2 = tab_srcs[w][:].rearrange("(n two) f -> n (two f)",
                                                    two=2)
                    for h in range(2):
                        win_ap[(w, h)] = tab2[:, h * H:(h + 1) * H]

                accs = {}
                done_groups = [0] * nblk
                if wb is not None:
                    agin, tabs, coll_chb, chb = wb
                    chunk_left = [chb[c + 1] - chb[c]
                                  for c in range(len(chb) - 1)]

                def emit_update(b):
                    ps2 = ps_upd.tile([H, P], F32, name="ps2", tag="ps2")
                    nc.tensor.matmul(out=ps2[:], lhsT=W_sb[:],
                                     rhs=macc[:, b * P:(b + 1) * P],
                                     start=True, stop=True)
                    tmp = mpool.tile([H, P], F32, name="tmp", tag="tmp")
                    nc.vector.tensor_tensor(out=tmp[:], in0=ps2[:],
                                            in1=hT[:, b * P:(b + 1) * P],
                                            op=AluOp.add)
                    nc.scalar.activation(out=hT[:, b * P:(b + 1) * P],
                                         in_=tmp[:], func=Act.Relu, bias=b_sb[:])
                    if wb is not None:
                        psr = ps_misc.tile([P, H], BF16, name="psr", tag="misc")
                        nc.tensor.transpose(out=psr[:],
                                            in_=hT[:, b * P:(b + 1) * P],
                                            identity=ident_sb[:])
                        nc.vector.tensor_copy(out=rstage[:, b * H:(b + 1) * H],
                                              in_=psr[:])
                        c = int(np.searchsorted(chb, b, "right")) - 1
                        chunk_left[c] -= 1
                        if chunk_left[c] == 0:
                            emit_chunk_dma(agin, chb[c], chb[c + 1])

                for ci, (w, h, t0, mu0, ulist) in enumerate(calls):
                    nt = sum(2 if units[u][1] else 1 for u in ulist)
                    nmu = len(ulist)
                    g = gpool.tile([P, TPC, H], BF16, name="g", tag="g")
                    _dma_gather_raw(
                        nc.gpsimd, g[:, :nt, :], win_ap[(w, h)],
                        idx_sb[:, t0 * 8:(t0 + nt) * 8],
                        num_idxs=nt * P, elem_size=H, elem_step=2 * H,
                        queue_num=ci % 4)
                    S = s_pool.tile([P, TPC, P], BF16, name="S", tag="S")
                    nc.vector.tensor_tensor(
                        out=S[:, :nmu, :],
                        in0=iota_sb[:, None, :].to_broadcast([P, nmu, P]),
                        in1=pdst_sb[:, mu0:mu0 + nmu, None]
                            .to_broadcast([P, nmu, P]),
                        op=AluOp.is_equal)
                    off = 0
                    for mi, u in enumerate(ulist):
                        grp, is_pack, _, first, last = units[u]
                        if is_pack:
                            ss = mpool.tile([P, H], BF16, name="ss", tag="ss",
                                            bufs=4)
                            nc.vector.tensor_tensor(
                                out=ss[:], in0=g[:, off, :], in1=g[:, off + 1, :],
                                op=AluOp.add)
                            lhs = ss[:]
                            off += 2
                        else:
                            lhs = g[:, off, :]
                            off += 1
                        if first:
                            accs[grp] = ps_acc.tile([H, P], F32, name="acc",
                                                    tag="acc")
                        nc.tensor.matmul(out=accs[grp][:], lhsT=lhs,
                                         rhs=S[:, mi, :], start=first, stop=last)
                        if last:
                            b = grp % nblk
                            glist = blk_groups[b]
                            if done_groups[b] == 0:
                                nc.vector.tensor_copy(
                                    out=macc[:, b * P:(b + 1) * P],
                                    in_=accs[grp][:])
                            else:
                                nc.vector.tensor_tensor(
                                    out=macc[:, b * P:(b + 1) * P],
                                    in0=macc[:, b * P:(b + 1) * P],
                                    in1=accs[grp][:], op=AluOp.add)
                            del accs[grp]
                            done_groups[b] += 1
                            if done_groups[b] == len(glist):
                                emit_update(b)
                if wb is not None:
                    emit_chunk_colls(agin, tabs, coll_chb)

            for r in range(ROUNDS):
                emit_phase(tabs_v[r], VWIN, idx_c_sb, pdst_c_sb,
                           calls_c, units_c, bg_c, NB_C, hcT, wv2c_sb, bv2c_sb,
                           (agin_c, tabs_c[r], CCH, CDM))
                last = r == ROUNDS - 1
                emit_phase(tabs_c[r], CWIN, idx_v_sb, pdst_v_sb,
                           calls_v, units_v, bg_v, NB_V, hvT, wc2v_sb, bc2v_sb,
                           None if last else
                           (agin_v, tabs_v[r + 1], VCH, VDM))

            # ---- scores = h_var @ w_score + b_score (shard)
            c0 = 0
            while c0 < V_S:
                wd = min(512, V_S - c0)
                pss = ps_misc.tile([1, 512], F32, name="pss", tag="misc")
                nc.tensor.matmul(out=pss[:, :wd], lhsT=wsco_sb[:],
                                 rhs=hvT[:, c0:c0 + wd], start=True, stop=True)
                sch = mpool.tile([1, 512], F32, name="sch", tag="sch")
                nc.vector.tensor_scalar(
                    out=sch[:, :wd], in0=pss[:, :wd],
                    scalar1=float(b_score_val), scalar2=None, op0=AluOp.add)
                nc.sync.dma_start(out=scores_out[None, c0:c0 + wd],
                                  in_=sch[0:1, :wd])
                c0 += wd

    nc.compile()
    return nc


_CACHE = {}


def kernel(**inputs):
    var_feat = np.asarray(inputs["var_feat"], np.float32)
    constr_feat = np.asarray(inputs["constr_feat"], np.float32)
    var_idx = np.asarray(inputs["var_idx"]).astype(np.int64)
    constr_idx = np.asarray(inputs["constr_idx"]).astype(np.int64)
    b_score_val = float(np.asarray(inputs["b_score"]).reshape(-1)[0])

    key = (var_idx.tobytes(), constr_idx.tobytes())
    if key in _CACHE:
        nc, idx_v, pdst_v, idx_c, pdst_c = _CACHE[key]
    else:
        rm_v = _row_map(V_CORE, VCH)
        rm_c = _row_map(C_CORE, CCH)
        # v2c: dst=constr, src=var
        idx_v, pdst_v, npk2_c, nraw_c = _prep_direction(
            constr_idx, rm_v[var_idx], C_CORE, NB_C, VWIN, RV // VWIN)
        # c2v: dst=var, src=constr
        idx_c, pdst_c, npk2_v, nraw_v = _prep_direction(
            var_idx, rm_c[constr_idx], V_CORE, NB_V, CWIN, RC // CWIN)
        nc = _build((npk2_c, nraw_c), (npk2_v, nraw_v), b_score_val)
        _CACHE[key] = (nc, idx_v, pdst_v, idx_c, pdst_c)

    iota = np.broadcast_to(np.arange(P, dtype=np.float32),
                           (P, P)).astype(BF).copy()
    ident = np.eye(H, dtype=np.float32).astype(BF)

    vf_pad = np.zeros((CORES, V_S, VF), np.float32)
    vf_pad[:, :V_CORE] = var_feat.reshape(CORES, V_CORE, VF)
    cf_pad = np.zeros((CORES, C_S, CF), np.float32)
    cf_pad[:, :C_CORE] = constr_feat.reshape(CORES, C_CORE, CF)

    common = dict(
        wvar=np.ascontiguousarray(inputs["W_var"], dtype=np.float32),
        wcon=np.ascontiguousarray(inputs["W_con"], dtype=np.float32),
        wv2c=np.ascontiguousarray(inputs["W_v2c"], dtype=np.float32),
        wc2v=np.ascontiguousarray(inputs["W_c2v"], dtype=np.float32),
        wsco=np.ascontiguousarray(inputs["W_score"], dtype=np.float32).astype(BF),
        bvar=np.ascontiguousarray(inputs["b_var"], dtype=np.float32).reshape(H, 1),
        bcon=np.ascontiguousarray(inputs["b_con"], dtype=np.float32).reshape(H, 1),
        bv2c=np.ascontiguousarray(inputs["b_v2c"], dtype=np.float32).reshape(H, 1),
        bc2v=np.ascontiguousarray(inputs["b_c2v"], dtype=np.float32).reshape(H, 1),
        iota=iota, ident=ident,
    )
    in_maps = []
    for k in range(CORES):
        m = dict(common)
        m["vfT"] = np.ascontiguousarray(vf_pad[k].T)
        m["cfT"] = np.ascontiguousarray(cf_pad[k].T)
        m["idx_v2c"] = idx_v[k]
        m["pdst_v2c"] = pdst_v[k]
        m["idx_c2v"] = idx_c[k]
        m["pdst_c2v"] = pdst_c[k]
        in_maps.append(m)

    res = run_bass_kernel_spmd(nc, in_maps, list(range(CORES)))
    scores = np.concatenate([res.results[k]["scores"].reshape(-1)[:V_CORE]
                             for k in range(CORES)])
    return scores.astype(np.float32)
